# revision 22
# baseline (speedup 1.0000x reference)
"""Trainium2 Bass kernel for nn_Block_7722351198258 (dual cross-attention block).

Sharding: 8 cores = 2 branches (S2T video-queries / T2S audio-queries) x 4 batch
elements. Each core runs an identical SPMD program computing one branch-batch:
LN -> (+pos, folded) -> 12-head cross-attention (1568 q x 1568 kv tokens) ->
output projection -> bottleneck adapter. The host folds LN gamma into the
projection weights, beta/pos/bias into precomputed additive tables, shards
inputs, and adds the raw residual + reassembles outputs at the end.

Device layout: post-LN activations are feature-major [768, tok] (PE-transposed),
so projections, attention (scores_T [kk, qq]), softmax (exp on ScalarE with the
denominator from a ones-column fused into the v-projection), and the adapter
all run as bf16 matmuls with K=128 (K=64 scores run as 2-head row-tiled pairs).
Stage C interleaves scores/exp/attn-v per 128-row kv chunk so PE and ACT
pipeline; softmax normalization is deferred to the PSUM evacuation.
"""

import numpy as np
import ml_dtypes

import concourse.bass as bass
import concourse.mybir as mybir
import concourse.tile as tile
from concourse.bass_utils import run_bass_kernel_spmd

F32 = mybir.dt.float32
BF16 = mybir.dt.bfloat16
AF = mybir.ActivationFunctionType
ALU = mybir.AluOpType

DIM = 768
H = 12
HD = 64
SCALE = HD ** -0.5
T = 8
TS = 8
NP = 196
B = 4
DOWN = 192
EPS = 1e-5

NQ = NP * T            # 1568 query tokens (= kv tokens)
NK = NQ
NEX = 8                # extra rows (CLS for S2T / dummies for T2S)
NQE = NQ + NEX         # 1576 adapter tokens
KD = DIM // 128        # 6 feature chunks
NKT = 13               # kv token tiles: 12*128 + 32
NQT = 13               # q-side token tiles: 12*128 + 40
QCH = 392              # qq chunk (4*392 = 1568)
ACH = 394              # adapter N chunk (4*394 = 1576)
VCH = 390              # v-projection N chunk (2*390 = 780)
DV = H * (HD + 1)      # 780: v with per-head ones column

MAX_WAITS = 1


def _split_excess_waits(nc):
    """The walrus build here rejects instructions with >1 sem-wait; split
    extras onto preceding same-engine EventSemaphore carriers."""
    n_fixed = 0
    for f in nc.m.functions:
        for b in f.blocks:
            out = []
            changed = False
            for inst in b.instructions:
                si = inst.sync_info
                if si is not None and si.on_wait is not None and len(si.on_wait) > MAX_WAITS:
                    waits = list(si.on_wait)
                    keep = waits[-MAX_WAITS:]
                    extra = waits[:-MAX_WAITS]
                    ci = 0
                    while extra:
                        chunk, extra = extra[:MAX_WAITS], extra[MAX_WAITS:]
                        out.append(mybir.InstEventSemaphore(
                            name=f"{inst.name}_waitfix{ci}", engine=inst.engine,
                            ins=[], outs=[],
                            sync_info=mybir.SyncInfo(on_wait=chunk, on_update=[])))
                        ci += 1
                    inst.sync_info = mybir.SyncInfo(
                        on_wait=keep, on_update=list(si.on_update or []))
                    changed = True
                    n_fixed += 1
                out.append(inst)
            if changed:
                b.instructions = out
    return n_fixed


def _qtile_rows(it):
    return 128 if it < NQT - 1 else NQE - 128 * (NQT - 1)  # 40


def _ktile_rows(it):
    return 128 if it < NKT - 1 else NK - 128 * (NKT - 1)   # 32


def build_nc():
    nc = bass.Bass(trn_type="TRN2", name="xattn_block")

    xq_d = nc.dram_tensor("xq", [NQE, DIM], F32, kind="ExternalInput")
    xkv_d = nc.dram_tensor("xkv", [NK, DIM], F32, kind="ExternalInput")
    wq_d = nc.dram_tensor("wq", [DIM, DIM], BF16, kind="ExternalInput")
    wk_d = nc.dram_tensor("wk", [DIM, DIM], BF16, kind="ExternalInput")
    wv_d = nc.dram_tensor("wv", [DIM, DV], BF16, kind="ExternalInput")
    posq_d = nc.dram_tensor("posq", [DIM, NQ], BF16, kind="ExternalInput")
    posk_d = nc.dram_tensor("posk", [DIM, NK], BF16, kind="ExternalInput")
    posv_d = nc.dram_tensor("posv", [NK, DV], BF16, kind="ExternalInput")
    wp_d = nc.dram_tensor("wp", [DIM, DIM], BF16, kind="ExternalInput")
    pb_d = nc.dram_tensor("pb", [DIM], F32, kind="ExternalInput")
    w1_d = nc.dram_tensor("w1", [DIM, DOWN], BF16, kind="ExternalInput")
    b1_d = nc.dram_tensor("b1", [256], F32, kind="ExternalInput")
    w2_d = nc.dram_tensor("w2", [256, DIM], BF16, kind="ExternalInput")
    b2_d = nc.dram_tensor("b2", [DIM], F32, kind="ExternalInput")
    gq_d = nc.dram_tensor("gq", [DIM], F32, kind="ExternalInput")
    bq_d = nc.dram_tensor("bq", [DIM], F32, kind="ExternalInput")
    ident_d = nc.dram_tensor("ident", [128, 128], F32, kind="ExternalInput")
    out_d = nc.dram_tensor("out", [DIM, NQE], F32, kind="ExternalOutput")

    wq_r = wq_d.rearrange("(kc p) d -> p kc d", p=128)
    wk_r = wk_d.rearrange("(kc p) d -> p kc d", p=128)
    wv_r = wv_d.rearrange("(kc p) d -> p kc d", p=128)
    wp_r = wp_d.rearrange("(kc p) d -> p kc d", p=128)
    w1_r = w1_d.rearrange("(kc p) d -> p kc d", p=128)
    w2_r = w2_d.rearrange("(kc p) d -> p kc d", p=128)

    with tile.TileContext(nc) as tc:
        with (
            tc.tile_pool(name="const", bufs=1) as const,
            tc.tile_pool(name="xnt", bufs=1) as xnt,
            tc.tile_pool(name="qkv", bufs=1) as qkv,
            tc.tile_pool(name="attn_out", bufs=1) as attn_out,
        ):
            eps_t = const.tile([128, 1], F32)
            nc.vector.memset(eps_t, EPS)
            ones64 = const.tile([1, 64], BF16)
            nc.vector.memset(ones64, 1.0)
            ident = const.tile([128, 128], F32)
            nc.sync.dma_start(out=ident, in_=ident_d[:])
            gq_sb = const.tile([128, KD], F32)
            nc.sync.dma_start(out=gq_sb, in_=gq_d.rearrange("(c p) -> p c", p=128))
            bq_sb = const.tile([128, KD], F32)
            nc.sync.dma_start(out=bq_sb, in_=bq_d.rearrange("(c p) -> p c", p=128))
            pb_sb = const.tile([128, KD], F32)
            nc.sync.dma_start(out=pb_sb, in_=pb_d.rearrange("(c p) -> p c", p=128))
            b1_sb = const.tile([128, 2], F32)
            nc.sync.dma_start(out=b1_sb, in_=b1_d.rearrange("(c p) -> p c", p=128))
            b2_sb = const.tile([128, KD], F32)
            nc.sync.dma_start(out=b2_sb, in_=b2_d.rearrange("(c p) -> p c", p=128))

            # feature-major normalized activations (post-LN, pre-gamma)
            xnT_q = xnt.tile([128, KD, NQE], BF16)
            xnT_kv = xnt.tile([128, KD, NK], BF16)

            q_T = qkv.tile([128, KD, NQ], BF16)
            k_T = qkv.tile([128, KD, NK], BF16)
            v_sb = qkv.tile([128, NKT, DV], BF16)

            O_T = attn_out.tile([128, KD, NQ], BF16)

            # ------------- stages B+C interleaved: projections + attention ---
            # One shared PSUM pool: tag "ps" [128,1024] (2 banks) x3 bufs used
            # by both projection chunks and score pairs; tag "ops" (1 bank) x2
            # for attn-v accumulators / broadcast tiles. Projection chunks for
            # q/k of head-pair mt are injected into attention pair mt-1's
            # ACT-bound stretches to keep PE dense.
            with (
                tc.tile_pool(name="wload", bufs=3) as wload,
                tc.tile_pool(name="pos", bufs=3) as pos,
                tc.tile_pool(name="cps", bufs=2, space="PSUM") as cps,
                tc.tile_pool(name="opsum", bufs=4, space="PSUM") as opsum,
                tc.tile_pool(name="exps", bufs=16) as exps,
                tc.tile_pool(name="norm", bufs=4) as norm,
            ):
                wv_sb = wload.tile([128, KD, DV], BF16, tag="wv", bufs=1)
                nc.sync.dma_start(out=wv_sb, in_=wv_r)
                w_cur = {}

                def emit_qk_chunk(name, mt, qc):
                    if qc == 0:
                        wt = wload.tile([128, KD, 128], BF16, tag="w", bufs=3,
                                        name=f"w_{name}{mt}")
                        w_r = wq_r if name == "q" else wk_r
                        nc.sync.dma_start(out=wt, in_=w_r[:, :, mt * 128:(mt + 1) * 128])
                        w_cur[name] = wt
                    w_sb = w_cur[name]
                    pos_d = posq_d if name == "q" else posk_d
                    outT = q_T if name == "q" else k_T
                    src = xnT_q if name == "q" else xnT_kv
                    n0 = qc * QCH
                    ps = cps.tile([128, 1024], F32, tag="ps", name=f"ps_{name}{mt}_{qc}")
                    for kc in range(KD):
                        nc.tensor.matmul(ps[:, 0:QCH],
                                         w_sb[:, kc, :],
                                         src[:, kc, n0:n0 + QCH],
                                         start=(kc == 0), stop=(kc == KD - 1))
                    pt = pos.tile([128, QCH], BF16, tag="pt")
                    nc.sync.dma_start(out=pt, in_=pos_d[mt * 128:(mt + 1) * 128,
                                                        n0:n0 + QCH])
                    nc.vector.tensor_add(out=outT[:, mt, n0:n0 + QCH],
                                         in0=ps[:, 0:QCH], in1=pt)

                def emit_v_chunk(tt, vc):
                    rows = _ktile_rows(tt)
                    t0 = tt * 128
                    n0 = vc * VCH
                    ps = cps.tile([128, 1024], F32, tag="ps", name=f"ps_v{tt}_{vc}")
                    for kc in range(KD):
                        nc.tensor.matmul(ps[:rows, 0:VCH], xnT_kv[:, kc, t0:t0 + rows],
                                         wv_sb[:, kc, n0:n0 + VCH],
                                         start=(kc == 0), stop=(kc == KD - 1))
                    pt = pos.tile([128, VCH], BF16, tag="ptv")
                    nc.sync.dma_start(out=pt[:rows], in_=posv_d[t0:t0 + rows, n0:n0 + VCH])
                    nc.vector.tensor_add(out=v_sb[:rows, tt, n0:n0 + VCH],
                                         in0=ps[:rows, 0:VCH], in1=pt[:rows])

                def emit_ln_tile(ln, side, it):
                    """LN one 128-token tile and PE-transpose it into
                    xnT_q/xnT_kv (transposes share the 'ps' PSUM slots; ACT
                    evacuates; the xn scale runs on the otherwise idle
                    GpSimd)."""
                    x_d, rows_of, xnT = ((xq_d, _qtile_rows, xnT_q) if side == "q"
                                         else (xkv_d, _ktile_rows, xnT_kv))
                    rows = rows_of(it)
                    r0 = it * 128
                    raw = ln.tile([128, DIM], F32, tag="raw", name=f"raw_{side}{it}")
                    nc.sync.dma_start(out=raw[:rows], in_=x_d[r0:r0 + rows, :])
                    stats = ln.tile([128, 3, 6], F32, tag="stats", name=f"st_{side}{it}")
                    rawg = raw.rearrange("p (s f) -> p s f", s=3)
                    for s in range(3):
                        nc.vector.bn_stats(out=stats[:rows, s, :], in_=rawg[:rows, s, :])
                    mv = ln.tile([128, 2], F32, tag="mv", name=f"mv_{side}{it}")
                    nc.vector.bn_aggr(out=mv[:rows], in_=stats[:rows])
                    std = ln.tile([128, 1], F32, tag="std", name=f"sd_{side}{it}")
                    nc.scalar.activation(out=std[:rows], in_=mv[:rows, 1:2],
                                         func=AF.Sqrt, bias=eps_t[:rows], scale=1.0)
                    rstd = ln.tile([128, 1], F32, tag="rstd", name=f"rs_{side}{it}")
                    nc.vector.reciprocal(out=rstd[:rows], in_=std[:rows])
                    xn = ln.tile([128, DIM], F32, tag="xn", name=f"xn_{side}{it}")
                    nc.gpsimd.tensor_scalar(
                        out=xn[:rows], in0=raw[:rows], scalar1=mv[:rows, 0:1],
                        scalar2=rstd[:rows], op0=ALU.subtract, op1=ALU.mult)
                    for c in range(KD):
                        tp = cps.tile([128, 1024], F32, tag="ps", name=f"tp_{side}{it}_{c}")
                        nc.tensor.transpose(tp[:, :rows], xn[:rows, c * 128:(c + 1) * 128],
                                            ident[:rows, :rows])
                        nc.scalar.copy(out=xnT[:, c, r0:r0 + rows], in_=tp[:, :rows])

                def emit_attnv(pr, q4, eS_tiles):
                    q0 = q4 * QCH
                    ops2 = []
                    for hi in range(2):
                        ops2.append(opsum.tile([65, QCH], F32, tag="ops",
                                               name=f"ops{pr}_{q4}_{hi}"))
                    for kc in range(NKT):
                        rows = _ktile_rows(kc)
                        for hi in range(2):
                            hh = 2 * pr + hi
                            nc.tensor.matmul(
                                ops2[hi], v_sb[:rows, kc, hh * 65:hh * 65 + 65],
                                eS_tiles[kc][:rows, hi, :],
                                start=(kc == 0), stop=(kc == NKT - 1))
                    return ops2

                def emit_norm(pr, q4, ops2):
                    q0 = q4 * QCH
                    for hi in range(2):
                        p0 = hi * 64
                        rinv = norm.tile([1, QCH], BF16, tag="rinv")
                        with nc.allow_low_precision(reason="softmax denom bf16 ok"):
                            nc.vector.reciprocal(out=rinv, in_=ops2[hi][64:65, :])
                        rbp = opsum.tile([65, QCH], F32, tag="ops",
                                         name=f"rbp{pr}_{q4}_{hi}")
                        nc.tensor.matmul(rbp[0:64, :], ones64, rinv)
                        rb = norm.tile([64, QCH], BF16, tag="rb")
                        nc.vector.tensor_copy(out=rb, in_=rbp[0:64, :])
                        nc.vector.tensor_mul(out=O_T[p0:p0 + 64, pr, q0:q0 + QCH],
                                             in0=ops2[hi][0:64, :], in1=rb)

                # stage A + prelude, interleaved for PE density: each kv tile's
                # LN/transpose immediately feeds its v-projection chunks; q
                # tiles feed pair-0 q/k projection chunks as token ranges
                # complete.
                import contextlib
                ln_stack = contextlib.ExitStack()
                ln = ln_stack.enter_context(tc.tile_pool(name="ln", bufs=4))
                for it in range(NKT):
                    emit_ln_tile(ln, "kv", it)
                    emit_v_chunk(it, 0)
                    emit_v_chunk(it, 1)
                qk0 = {3: [("q", 0, 0), ("k", 0, 0)], 6: [("q", 0, 1), ("k", 0, 1)],
                       9: [("q", 0, 2), ("k", 0, 2)],
                       12: [("q", 0, 3), ("k", 0, 3)]}
                for it in range(NQT):
                    emit_ln_tile(ln, "q", it)
                    for ch in qk0.get(it, []):
                        emit_qk_chunk(*ch)
                ln_stack.close()

                # software-pipelined units: unit u's scores/exp interleave with
                # unit u-1's attn-v, chunk by chunk; next pair's projection
                # chunks drop into the leftover ACT-bound slack
                units = [(pr, q4) for pr in range(KD) for q4 in range(4)]
                prev = None      # (pr, q4, eS_tiles)
                inject = []
                for pr, q4 in units:
                    if q4 == 0 and pr + 1 < KD:
                        inject = [("q", pr + 1, qc) for qc in range(4)] + \
                                 [("k", pr + 1, qc) for qc in range(4)]
                    q0 = q4 * QCH
                    eS_tiles = []
                    prev_ops = None
                    if prev is not None:
                        prev_ops = []
                        for hi in range(2):
                            prev_ops.append(opsum.tile(
                                [65, QCH], F32, tag="ops",
                                name=f"ops{prev[0]}_{prev[1]}_{hi}"))
                    for kc in range(NKT):
                        rows = _ktile_rows(kc)
                        k0 = kc * 128
                        psQ = cps.tile([128, 1024], F32, tag="ps",
                                       name=f"psQ{pr}_{q4}_{kc}")
                        nc.tensor.matmul(psQ[:rows, 0:QCH],
                                         k_T[0:64, pr, k0:k0 + rows],
                                         q_T[0:64, pr, q0:q0 + QCH],
                                         tile_position=(0, 0))
                        nc.tensor.matmul(psQ[:rows, 512:512 + QCH],
                                         k_T[64:128, pr, k0:k0 + rows],
                                         q_T[64:128, pr, q0:q0 + QCH],
                                         tile_position=(64, 0))
                        eS = exps.tile([128, 2, QCH], BF16, tag="eS",
                                       name=f"eS{pr}_{q4}_{kc}")
                        psv = psQ.rearrange("p (h q) -> p h q", h=2)
                        nc.scalar.activation(out=eS[:rows], in_=psv[:rows, :, 0:QCH],
                                             func=AF.Exp)
                        eS_tiles.append(eS)
                        if prev is not None:
                            prows = _ktile_rows(kc)
                            for hi in range(2):
                                hh = 2 * prev[0] + hi
                                nc.tensor.matmul(
                                    prev_ops[hi],
                                    v_sb[:prows, kc, hh * 65:hh * 65 + 65],
                                    prev[2][kc][:prows, hi, :],
                                    start=(kc == 0), stop=(kc == NKT - 1))
                        if kc in (4, 9) and inject:
                            emit_qk_chunk(*inject.pop(0))
                    if prev is not None:
                        emit_norm(prev[0], prev[1], prev_ops)
                    prev = (pr, q4, eS_tiles)
                # drain the pipeline: last unit's attn-v + norm
                ops2 = emit_attnv(prev[0], prev[1], prev[2])
                emit_norm(prev[0], prev[1], ops2)

            # ---------------- stage D: output projection (+CLS LN) ----------------
            xatt_T = attn_out.tile([128, KD, NQE], BF16)
            with (
                tc.tile_pool(name="wp_pool", bufs=1) as wp_pool,
                tc.tile_pool(name="dps", bufs=3, space="PSUM") as dps,
            ):
                wp_sb = wp_pool.tile([128, KD, DIM], BF16)
                nc.sync.dma_start(out=wp_sb, in_=wp_r)
                for mt in range(KD):
                    for qc in range(4):
                        n0 = qc * QCH
                        ps = dps.tile([128, QCH], F32, tag="dps")
                        for kc in range(KD):
                            nc.tensor.matmul(ps, wp_sb[:, kc, mt * 128:(mt + 1) * 128],
                                             O_T[:, kc, n0:n0 + QCH],
                                             start=(kc == 0), stop=(kc == KD - 1))
                        nc.vector.tensor_scalar_add(
                            out=xatt_T[:, mt, n0:n0 + QCH], in0=ps,
                            scalar1=pb_sb[:, mt:mt + 1])
                    # CLS columns: full LN = xn*g + b
                    nc.vector.tensor_scalar(
                        out=xatt_T[:, mt, NQ:NQE], in0=xnT_q[:, mt, NQ:NQE],
                        scalar1=gq_sb[:, mt:mt + 1], scalar2=bq_sb[:, mt:mt + 1],
                        op0=ALU.mult, op1=ALU.add)

            # ---------------- stage E/F: adapter ----------------
            with (
                tc.tile_pool(name="ad_w", bufs=1) as ad_w,
                tc.tile_pool(name="gelu", bufs=1) as gelu,
                tc.tile_pool(name="aps", bufs=3, space="PSUM") as aps,
                tc.tile_pool(name="fin", bufs=3) as fin,
            ):
                w1_sb = ad_w.tile([128, KD, DOWN], BF16)
                nc.sync.dma_start(out=w1_sb, in_=w1_r)
                w2_sb = ad_w.tile([128, 2, DIM], BF16)
                nc.sync.dma_start(out=w2_sb, in_=w2_r)
                g_T = gelu.tile([128, 2, NQE], BF16)
                for mt2, msz in ((0, 128), (1, 64)):
                    for qc in range(4):
                        n0 = qc * ACH
                        ps = aps.tile([128, ACH], F32, tag="aps1")
                        for kc in range(KD):
                            nc.tensor.matmul(ps[:msz], w1_sb[:, kc, mt2 * 128:mt2 * 128 + msz],
                                             xatt_T[:, kc, n0:n0 + ACH],
                                             start=(kc == 0), stop=(kc == KD - 1))
                        nc.scalar.activation(out=g_T[:msz, mt2, n0:n0 + ACH], in_=ps[:msz],
                                             func=AF.Gelu, bias=b1_sb[:msz, mt2:mt2 + 1],
                                             scale=1.0)
                for mt in range(KD):
                    for qc in range(4):
                        n0 = qc * ACH
                        ps = aps.tile([128, ACH], F32, tag="aps2")
                        nc.tensor.matmul(ps, w2_sb[:, 0, mt * 128:(mt + 1) * 128],
                                         g_T[:, 0, n0:n0 + ACH], start=True, stop=False)
                        nc.tensor.matmul(ps, w2_sb[0:64, 1, mt * 128:(mt + 1) * 128],
                                         g_T[0:64, 1, n0:n0 + ACH], start=False, stop=True)
                        # final = mlp + x_att (+ b2); host adds the raw residual
                        ft = fin.tile([128, ACH], F32, tag="ft")
                        nc.vector.tensor_scalar_add(out=ft, in0=ps,
                                                    scalar1=b2_sb[:, mt:mt + 1])
                        nc.vector.tensor_add(out=ft, in0=ft, in1=xatt_T[:, mt, n0:n0 + ACH])
                        nc.sync.dma_start(out=out_d[mt * 128:(mt + 1) * 128, n0:n0 + ACH],
                                          in_=ft)

    return nc


_NC_CACHE = None
_NC_FIXED = False


def _get_nc(fixed=False):
    """fixed=True applies the walrus wait-split (breaks CoreSim bookkeeping,
    required for NEFF compile)."""
    global _NC_CACHE, _NC_FIXED
    if _NC_CACHE is None:
        _NC_CACHE = build_nc()
    if fixed and not _NC_FIXED:
        _split_excess_waits(_NC_CACHE)
        _NC_FIXED = True
    return _NC_CACHE


def _prep_branch(params, branch):
    """Precompute per-branch device weights/tables (host-side folding)."""
    p = {k: np.asarray(v, np.float32) for k, v in params.items()}
    if branch == "s2t":
        pre = "s2t"
        gq, bq = p["ln_t_g"], p["ln_t_b"]          # q side = video stream
        gkv, bkv = p["ln_s_g"], p["ln_s_b"]        # kv side = audio stream
        space_q, temp_q = p[f"{pre}_vmae_space"], p[f"{pre}_vmae_temp"]
        space_k, temp_k = p[f"{pre}_clip_space"], p[f"{pre}_clip_temp"]
        ad = "ad_t"
    else:
        pre = "t2s"
        gq, bq = p["ln_s_g"], p["ln_s_b"]
        gkv, bkv = p["ln_t_g"], p["ln_t_b"]
        space_q, temp_q = p[f"{pre}_clip_space"], p[f"{pre}_clip_temp"]
        space_k, temp_k = p[f"{pre}_vmae_space"], p[f"{pre}_vmae_temp"]
        ad = "ad_s"

    qw, qb = p[f"{pre}_qw"], p[f"{pre}_qb"]
    kvw, kvb = p[f"{pre}_kvw"], p[f"{pre}_kvb"]
    kw, vw = kvw[:DIM], kvw[DIM:]
    kb, vb = kvb[:DIM], kvb[DIM:]
    pw, pb = p[f"{pre}_pw"], p[f"{pre}_pb"]
    w1, b1 = p[f"{ad}_w1"], p[f"{ad}_b1"]
    w2, b2 = p[f"{ad}_w2"], p[f"{ad}_b2"]

    # pos tables in attention token order (n-major, t-minor)
    pos_q = (space_q[:, None, :] + temp_q[None, :, :]).reshape(NQ, DIM)
    pos_k = (space_k[:, None, :] + temp_k[None, :, :]).reshape(NK, DIM)

    bf = lambda x: np.ascontiguousarray(x).astype(ml_dtypes.bfloat16)
    f32 = lambda x: np.ascontiguousarray(x, np.float32)

    Wq = (gq[:, None] * qw.T * SCALE)                       # [din, dout]
    posq_proj = ((bq + pos_q) @ qw.T + qb) * SCALE          # [NQ, dout]
    Wk = (gkv[:, None] * kw.T)
    posk_proj = (bkv + pos_k) @ kw.T + kb
    Wv = (gkv[:, None] * vw.T)                              # [din, 768]
    posv_proj = (bkv + pos_k) @ vw.T + vb                   # [NK, 768]

    Wv_aug = np.zeros((DIM, DV), np.float32)
    posv_aug = np.zeros((NK, DV), np.float32)
    for h in range(H):
        Wv_aug[:, h * 65:h * 65 + 64] = Wv[:, h * 64:(h + 1) * 64]
        posv_aug[:, h * 65:h * 65 + 64] = posv_proj[:, h * 64:(h + 1) * 64]
        posv_aug[:, h * 65 + 64] = 1.0

    b1_pad = np.zeros(256, np.float32)
    b1_pad[:DOWN] = b1
    w2_pad = np.zeros((256, DIM), np.float32)
    w2_pad[:DOWN] = w2.T

    return {
        "wq": bf(Wq), "wk": bf(Wk), "wv": bf(Wv_aug),
        "posq": bf(posq_proj.T), "posk": bf(posk_proj.T), "posv": bf(posv_aug),
        "wp": bf(pw.T), "pb": f32(pb),
        "w1": bf(w1.T), "b1": f32(b1_pad), "w2": bf(w2_pad), "b2": f32(b2),
        "gq": f32(gq), "bq": f32(bq),
        "ident": f32(np.eye(128, dtype=np.float32)),
    }


def kernel(s_x, t_x, params):
    s_x = np.asarray(s_x, np.float32)
    t_x = np.asarray(t_x, np.float32)
    nc = _get_nc(fixed=True)

    branch_inputs = {b: _prep_branch(params, b) for b in ("s2t", "t2s")}
    in_maps = []
    for c in range(8):
        branch = "s2t" if c < 4 else "t2s"
        b = c % 4
        bsl = slice(b * T, (b + 1) * T)
        m = dict(branch_inputs[branch])
        if branch == "s2t":
            xq = np.concatenate([
                np.ascontiguousarray(t_x[1:, bsl]).reshape(NQ, DIM),
                np.ascontiguousarray(t_x[0, bsl]),
            ], axis=0)
            xkv = np.ascontiguousarray(s_x[:, bsl]).reshape(NK, DIM)
        else:
            xq = np.concatenate([
                np.ascontiguousarray(s_x[:, bsl]).reshape(NQ, DIM),
                np.zeros((NEX, DIM), np.float32),
            ], axis=0)
            xkv = np.ascontiguousarray(t_x[1:, bsl]).reshape(NK, DIM)
        m["xq"] = np.ascontiguousarray(xq, np.float32)
        m["xkv"] = np.ascontiguousarray(xkv, np.float32)
        in_maps.append(m)

    res = run_bass_kernel_spmd(nc, in_maps, core_ids=list(range(8)))

    s_out = s_x.copy().reshape(NP, B, TS, DIM)
    t_out = t_x.copy().reshape(1 + NP, B, T, DIM)
    for c in range(8):
        o = np.asarray(res.results[c]["out"], np.float32).T  # [1576, 768]
        b = c % 4
        if c < 4:
            t_out[1:, b] += o[:NQ].reshape(NP, T, DIM)
            t_out[0, b] += o[NQ:NQE]
        else:
            s_out[:, b] += o[:NQ].reshape(NP, TS, DIM)
    return (s_out.reshape(NP, B * TS, DIM).astype(np.float32),
            t_out.reshape(1 + NP, B * T, DIM).astype(np.float32))


# revision 23
# speedup vs baseline: 1.2599x; 1.2599x over previous
"""Trainium2 Bass kernel for nn_Block_7722351198258 (dual cross-attention block).

Sharding: 8 cores = 2 branches (S2T video-queries / T2S audio-queries) x 4 batch
elements. Each core runs an identical SPMD program computing one branch-batch:
LN -> (+pos, folded) -> 12-head cross-attention (1568 q x 1568 kv tokens) ->
output projection -> bottleneck adapter. The host folds LN gamma into the
projection weights, beta/pos/bias into precomputed additive tables, shards
inputs, and adds the raw residual + reassembles outputs at the end.

Device layout: post-LN activations are feature-major [768, tok] (PE-transposed),
so projections, attention (scores_T [kk, qq]), softmax (exp on ScalarE with the
denominator from a ones-column fused into the v-projection), and the adapter
all run as bf16 matmuls with K=128 (K=64 scores run as 2-head row-tiled pairs).
Stage C interleaves scores/exp/attn-v per 128-row kv chunk so PE and ACT
pipeline; softmax normalization is deferred to the PSUM evacuation.
"""

import numpy as np
import ml_dtypes

import concourse.bass as bass
import concourse.mybir as mybir
import concourse.tile as tile
from concourse.bass_utils import run_bass_kernel_spmd

F32 = mybir.dt.float32
BF16 = mybir.dt.bfloat16
AF = mybir.ActivationFunctionType
ALU = mybir.AluOpType

DIM = 768
H = 12
HD = 64
SCALE = HD ** -0.5
T = 8
TS = 8
NP = 196
B = 4
DOWN = 192
EPS = 1e-5

NQ = NP * T            # 1568 query tokens (= kv tokens)
NK = NQ
NEX = 8                # extra rows (CLS for S2T / dummies for T2S)
NQE = NQ + NEX         # 1576 adapter tokens
KD = DIM // 128        # 6 feature chunks
NKT = 13               # kv token tiles: 12*128 + 32
NQT = 13               # q-side token tiles: 12*128 + 40
QCH = 392              # qq chunk (4*392 = 1568)
ACH = 394              # adapter N chunk (4*394 = 1576)
VCH = 390              # v-projection N chunk (2*390 = 780)
DV = H * (HD + 1)      # 780: v with per-head ones column

MAX_WAITS = 1


def _split_excess_waits(nc):
    """The walrus build here rejects instructions with >1 sem-wait; split
    extras onto preceding same-engine EventSemaphore carriers."""
    n_fixed = 0
    for f in nc.m.functions:
        for b in f.blocks:
            out = []
            changed = False
            for inst in b.instructions:
                si = inst.sync_info
                if si is not None and si.on_wait is not None and len(si.on_wait) > MAX_WAITS:
                    waits = list(si.on_wait)
                    keep = waits[-MAX_WAITS:]
                    extra = waits[:-MAX_WAITS]
                    ci = 0
                    while extra:
                        chunk, extra = extra[:MAX_WAITS], extra[MAX_WAITS:]
                        out.append(mybir.InstEventSemaphore(
                            name=f"{inst.name}_waitfix{ci}", engine=inst.engine,
                            ins=[], outs=[],
                            sync_info=mybir.SyncInfo(on_wait=chunk, on_update=[])))
                        ci += 1
                    inst.sync_info = mybir.SyncInfo(
                        on_wait=keep, on_update=list(si.on_update or []))
                    changed = True
                    n_fixed += 1
                out.append(inst)
            if changed:
                b.instructions = out
    return n_fixed


def _qtile_rows(it):
    return 128 if it < NQT - 1 else NQE - 128 * (NQT - 1)  # 40


def _ktile_rows(it):
    return 128 if it < NKT - 1 else NK - 128 * (NKT - 1)   # 32


def build_nc():
    nc = bass.Bass(trn_type="TRN2", name="xattn_block")

    xq_d = nc.dram_tensor("xq", [NQE, DIM], F32, kind="ExternalInput")
    xkv_d = nc.dram_tensor("xkv", [NK, DIM], F32, kind="ExternalInput")
    wq_d = nc.dram_tensor("wq", [DIM, DIM], BF16, kind="ExternalInput")
    wk_d = nc.dram_tensor("wk", [DIM, DIM], BF16, kind="ExternalInput")
    wv_d = nc.dram_tensor("wv", [DIM, DV], BF16, kind="ExternalInput")
    posq_d = nc.dram_tensor("posq", [DIM, NQ], BF16, kind="ExternalInput")
    posk_d = nc.dram_tensor("posk", [DIM, NK], BF16, kind="ExternalInput")
    posv_d = nc.dram_tensor("posv", [NK, DV], BF16, kind="ExternalInput")
    wp_d = nc.dram_tensor("wp", [DIM, DIM], BF16, kind="ExternalInput")
    pb_d = nc.dram_tensor("pb", [DIM], F32, kind="ExternalInput")
    w1_d = nc.dram_tensor("w1", [DIM, DOWN], BF16, kind="ExternalInput")
    b1_d = nc.dram_tensor("b1", [256], F32, kind="ExternalInput")
    w2_d = nc.dram_tensor("w2", [256, DIM], BF16, kind="ExternalInput")
    b2_d = nc.dram_tensor("b2", [DIM], F32, kind="ExternalInput")
    gq_d = nc.dram_tensor("gq", [DIM], F32, kind="ExternalInput")
    bq_d = nc.dram_tensor("bq", [DIM], F32, kind="ExternalInput")
    ident_d = nc.dram_tensor("ident", [128, 128], F32, kind="ExternalInput")
    out_d = nc.dram_tensor("out", [DIM, NQE], F32, kind="ExternalOutput")

    wq_r = wq_d.rearrange("(kc p) d -> p kc d", p=128)
    wk_r = wk_d.rearrange("(kc p) d -> p kc d", p=128)
    wv_r = wv_d.rearrange("(kc p) d -> p kc d", p=128)
    wp_r = wp_d.rearrange("(kc p) d -> p kc d", p=128)
    w1_r = w1_d.rearrange("(kc p) d -> p kc d", p=128)
    w2_r = w2_d.rearrange("(kc p) d -> p kc d", p=128)

    with tile.TileContext(nc) as tc:
        with (
            tc.tile_pool(name="const", bufs=1) as const,
            tc.tile_pool(name="xnt", bufs=1) as xnt,
            tc.tile_pool(name="qkv", bufs=1) as qkv,
            tc.tile_pool(name="attn_out", bufs=1) as attn_out,
        ):
            eps_t = const.tile([128, 1], F32)
            nc.vector.memset(eps_t, EPS)
            ones64 = const.tile([1, 64], BF16)
            nc.vector.memset(ones64, 1.0)
            ident = const.tile([128, 128], F32)
            nc.sync.dma_start(out=ident, in_=ident_d[:])
            gq_sb = const.tile([128, KD], F32)
            nc.sync.dma_start(out=gq_sb, in_=gq_d.rearrange("(c p) -> p c", p=128))
            bq_sb = const.tile([128, KD], F32)
            nc.sync.dma_start(out=bq_sb, in_=bq_d.rearrange("(c p) -> p c", p=128))
            pb_sb = const.tile([128, KD], F32)
            nc.sync.dma_start(out=pb_sb, in_=pb_d.rearrange("(c p) -> p c", p=128))
            b1_sb = const.tile([128, 2], F32)
            nc.sync.dma_start(out=b1_sb, in_=b1_d.rearrange("(c p) -> p c", p=128))
            b2_sb = const.tile([128, KD], F32)
            nc.sync.dma_start(out=b2_sb, in_=b2_d.rearrange("(c p) -> p c", p=128))

            # feature-major normalized activations (post-LN, pre-gamma)
            xnT_q = xnt.tile([128, KD, NQE], BF16)
            xnT_kv = xnt.tile([128, KD, NK], BF16)

            q_T = qkv.tile([128, KD, NQ], BF16)
            k_T = qkv.tile([128, KD, NK], BF16)
            v_sb = qkv.tile([128, NKT, DV], BF16)

            O_T = attn_out.tile([128, KD, NQ], BF16)

            # ------------- stages B+C interleaved: projections + attention ---
            # One shared PSUM pool: tag "ps" [128,1024] (2 banks) x3 bufs used
            # by both projection chunks and score pairs; tag "ops" (1 bank) x2
            # for attn-v accumulators / broadcast tiles. Projection chunks for
            # q/k of head-pair mt are injected into attention pair mt-1's
            # ACT-bound stretches to keep PE dense.
            with (
                tc.tile_pool(name="wload", bufs=3) as wload,
                tc.tile_pool(name="pos", bufs=3) as pos,
                tc.tile_pool(name="cps", bufs=2, space="PSUM") as cps,
                tc.tile_pool(name="opsum", bufs=4, space="PSUM") as opsum,
                tc.tile_pool(name="exps", bufs=16) as exps,
                tc.tile_pool(name="norm", bufs=4) as norm,
            ):
                wv_sb = wload.tile([128, KD, DV], BF16, tag="wv", bufs=1)
                nc.sync.dma_start(out=wv_sb, in_=wv_r)
                w_cur = {}

                def emit_qk_chunk(name, mt, qc):
                    if qc == 0:
                        wt = wload.tile([128, KD, 128], BF16, tag="w", bufs=3,
                                        name=f"w_{name}{mt}")
                        w_r = wq_r if name == "q" else wk_r
                        nc.sync.dma_start(out=wt, in_=w_r[:, :, mt * 128:(mt + 1) * 128])
                        w_cur[name] = wt
                    w_sb = w_cur[name]
                    pos_d = posq_d if name == "q" else posk_d
                    outT = q_T if name == "q" else k_T
                    src = xnT_q if name == "q" else xnT_kv
                    n0 = qc * QCH
                    ps = cps.tile([128, 1024], F32, tag="ps", name=f"ps_{name}{mt}_{qc}")
                    for kc in range(KD):
                        nc.tensor.matmul(ps[:, 0:QCH],
                                         w_sb[:, kc, :],
                                         src[:, kc, n0:n0 + QCH],
                                         start=(kc == 0), stop=(kc == KD - 1))
                    pt = pos.tile([128, QCH], BF16, tag="pt")
                    nc.sync.dma_start(out=pt, in_=pos_d[mt * 128:(mt + 1) * 128,
                                                        n0:n0 + QCH])
                    nc.vector.tensor_add(out=outT[:, mt, n0:n0 + QCH],
                                         in0=ps[:, 0:QCH], in1=pt)

                def emit_v_chunk(tt, vc):
                    rows = _ktile_rows(tt)
                    t0 = tt * 128
                    n0 = vc * VCH
                    ps = cps.tile([128, 1024], F32, tag="ps", name=f"ps_v{tt}_{vc}")
                    for kc in range(KD):
                        nc.tensor.matmul(ps[:rows, 0:VCH], xnT_kv[:, kc, t0:t0 + rows],
                                         wv_sb[:, kc, n0:n0 + VCH],
                                         start=(kc == 0), stop=(kc == KD - 1))
                    pt = pos.tile([128, VCH], BF16, tag="ptv")
                    nc.sync.dma_start(out=pt[:rows], in_=posv_d[t0:t0 + rows, n0:n0 + VCH])
                    nc.vector.tensor_add(out=v_sb[:rows, tt, n0:n0 + VCH],
                                         in0=ps[:rows, 0:VCH], in1=pt[:rows])

                def emit_ln_tile(ln, side, it):
                    """LN one 128-token tile and PE-transpose it into
                    xnT_q/xnT_kv (transposes share the 'ps' PSUM slots; ACT
                    evacuates; the xn scale runs on the otherwise idle
                    GpSimd)."""
                    x_d, rows_of, xnT = ((xq_d, _qtile_rows, xnT_q) if side == "q"
                                         else (xkv_d, _ktile_rows, xnT_kv))
                    rows = rows_of(it)
                    r0 = it * 128
                    raw = ln.tile([128, DIM], F32, tag="raw", name=f"raw_{side}{it}")
                    nc.sync.dma_start(out=raw[:rows], in_=x_d[r0:r0 + rows, :])
                    stats = ln.tile([128, 3, 6], F32, tag="stats", name=f"st_{side}{it}")
                    rawg = raw.rearrange("p (s f) -> p s f", s=3)
                    for s in range(3):
                        nc.vector.bn_stats(out=stats[:rows, s, :], in_=rawg[:rows, s, :])
                    mv = ln.tile([128, 2], F32, tag="mv", name=f"mv_{side}{it}")
                    nc.vector.bn_aggr(out=mv[:rows], in_=stats[:rows])
                    std = ln.tile([128, 1], F32, tag="std", name=f"sd_{side}{it}")
                    nc.scalar.activation(out=std[:rows], in_=mv[:rows, 1:2],
                                         func=AF.Sqrt, bias=eps_t[:rows], scale=1.0)
                    rstd = ln.tile([128, 1], F32, tag="rstd", name=f"rs_{side}{it}")
                    nc.vector.reciprocal(out=rstd[:rows], in_=std[:rows])
                    xn = ln.tile([128, DIM], F32, tag="xn", name=f"xn_{side}{it}")
                    nc.vector.tensor_scalar(
                        out=xn[:rows], in0=raw[:rows], scalar1=mv[:rows, 0:1],
                        scalar2=rstd[:rows], op0=ALU.subtract, op1=ALU.mult)
                    for c in range(KD):
                        tp = cps.tile([128, 1024], F32, tag="ps", name=f"tp_{side}{it}_{c}")
                        nc.tensor.transpose(tp[:, :rows], xn[:rows, c * 128:(c + 1) * 128],
                                            ident[:rows, :rows])
                        nc.scalar.copy(out=xnT[:, c, r0:r0 + rows], in_=tp[:, :rows])

                def emit_attnv(pr, q4, eS_tiles):
                    q0 = q4 * QCH
                    ops2 = []
                    for hi in range(2):
                        ops2.append(opsum.tile([65, QCH], F32, tag="ops",
                                               name=f"ops{pr}_{q4}_{hi}"))
                    for kc in range(NKT):
                        rows = _ktile_rows(kc)
                        for hi in range(2):
                            hh = 2 * pr + hi
                            nc.tensor.matmul(
                                ops2[hi], v_sb[:rows, kc, hh * 65:hh * 65 + 65],
                                eS_tiles[kc][:rows, hi, :],
                                start=(kc == 0), stop=(kc == NKT - 1))
                    return ops2

                def emit_norm(pr, q4, ops2):
                    q0 = q4 * QCH
                    for hi in range(2):
                        p0 = hi * 64
                        rinv = norm.tile([1, QCH], BF16, tag="rinv")
                        with nc.allow_low_precision(reason="softmax denom bf16 ok"):
                            nc.vector.reciprocal(out=rinv, in_=ops2[hi][64:65, :])
                        rbp = opsum.tile([65, QCH], F32, tag="ops",
                                         name=f"rbp{pr}_{q4}_{hi}")
                        nc.tensor.matmul(rbp[0:64, :], ones64, rinv)
                        rb = norm.tile([64, QCH], BF16, tag="rb")
                        nc.vector.tensor_copy(out=rb, in_=rbp[0:64, :])
                        nc.vector.tensor_mul(out=O_T[p0:p0 + 64, pr, q0:q0 + QCH],
                                             in0=ops2[hi][0:64, :], in1=rb)

                # stage A + prelude, interleaved for PE density: each kv tile's
                # LN/transpose immediately feeds its v-projection chunks; q
                # tiles feed pair-0 q/k projection chunks as token ranges
                # complete.
                import contextlib
                ln_stack = contextlib.ExitStack()
                ln = ln_stack.enter_context(tc.tile_pool(name="ln", bufs=4))
                for it in range(NKT):
                    emit_ln_tile(ln, "kv", it)
                    emit_v_chunk(it, 0)
                    emit_v_chunk(it, 1)
                qk0 = {3: [("q", 0, 0), ("k", 0, 0)], 6: [("q", 0, 1), ("k", 0, 1)],
                       9: [("q", 0, 2), ("k", 0, 2)],
                       12: [("q", 0, 3), ("k", 0, 3)]}
                for it in range(NQT):
                    emit_ln_tile(ln, "q", it)
                    for ch in qk0.get(it, []):
                        emit_qk_chunk(*ch)
                ln_stack.close()

                # software-pipelined units: unit u's scores/exp interleave with
                # unit u-1's attn-v, chunk by chunk; next pair's projection
                # chunks drop into the leftover ACT-bound slack
                units = [(pr, q4) for pr in range(KD) for q4 in range(4)]
                prev = None      # (pr, q4, eS_tiles)
                inject = []
                for pr, q4 in units:
                    if q4 == 0 and pr + 1 < KD:
                        inject = [("q", pr + 1, qc) for qc in range(4)] + \
                                 [("k", pr + 1, qc) for qc in range(4)]
                    q0 = q4 * QCH
                    eS_tiles = []
                    prev_ops = None
                    if prev is not None:
                        prev_ops = []
                        for hi in range(2):
                            prev_ops.append(opsum.tile(
                                [65, QCH], F32, tag="ops",
                                name=f"ops{prev[0]}_{prev[1]}_{hi}"))
                    for kc in range(NKT):
                        rows = _ktile_rows(kc)
                        k0 = kc * 128
                        psQ = cps.tile([128, 1024], F32, tag="ps",
                                       name=f"psQ{pr}_{q4}_{kc}")
                        nc.tensor.matmul(psQ[:rows, 0:QCH],
                                         k_T[0:64, pr, k0:k0 + rows],
                                         q_T[0:64, pr, q0:q0 + QCH],
                                         tile_position=(0, 0))
                        nc.tensor.matmul(psQ[:rows, 512:512 + QCH],
                                         k_T[64:128, pr, k0:k0 + rows],
                                         q_T[64:128, pr, q0:q0 + QCH],
                                         tile_position=(64, 0))
                        eS = exps.tile([128, 2, QCH], BF16, tag="eS",
                                       name=f"eS{pr}_{q4}_{kc}")
                        psv = psQ.rearrange("p (h q) -> p h q", h=2)
                        nc.scalar.activation(out=eS[:rows], in_=psv[:rows, :, 0:QCH],
                                             func=AF.Exp)
                        eS_tiles.append(eS)
                        if prev is not None:
                            prows = _ktile_rows(kc)
                            for hi in range(2):
                                hh = 2 * prev[0] + hi
                                nc.tensor.matmul(
                                    prev_ops[hi],
                                    v_sb[:prows, kc, hh * 65:hh * 65 + 65],
                                    prev[2][kc][:prows, hi, :],
                                    start=(kc == 0), stop=(kc == NKT - 1))
                        if kc in (4, 9) and inject:
                            emit_qk_chunk(*inject.pop(0))
                    if prev is not None:
                        emit_norm(prev[0], prev[1], prev_ops)
                    prev = (pr, q4, eS_tiles)
                # drain the pipeline: last unit's attn-v + norm
                ops2 = emit_attnv(prev[0], prev[1], prev[2])
                emit_norm(prev[0], prev[1], ops2)

            # ---------------- stage D: output projection (+CLS LN) ----------------
            xatt_T = attn_out.tile([128, KD, NQE], BF16)
            with (
                tc.tile_pool(name="wp_pool", bufs=1) as wp_pool,
                tc.tile_pool(name="dps", bufs=3, space="PSUM") as dps,
            ):
                wp_sb = wp_pool.tile([128, KD, DIM], BF16)
                nc.sync.dma_start(out=wp_sb, in_=wp_r)
                for mt in range(KD):
                    for qc in range(4):
                        n0 = qc * QCH
                        ps = dps.tile([128, QCH], F32, tag="dps")
                        for kc in range(KD):
                            nc.tensor.matmul(ps, wp_sb[:, kc, mt * 128:(mt + 1) * 128],
                                             O_T[:, kc, n0:n0 + QCH],
                                             start=(kc == 0), stop=(kc == KD - 1))
                        nc.vector.tensor_scalar_add(
                            out=xatt_T[:, mt, n0:n0 + QCH], in0=ps,
                            scalar1=pb_sb[:, mt:mt + 1])
                    # CLS columns: full LN = xn*g + b
                    nc.vector.tensor_scalar(
                        out=xatt_T[:, mt, NQ:NQE], in0=xnT_q[:, mt, NQ:NQE],
                        scalar1=gq_sb[:, mt:mt + 1], scalar2=bq_sb[:, mt:mt + 1],
                        op0=ALU.mult, op1=ALU.add)

            # ---------------- stage E/F: adapter ----------------
            with (
                tc.tile_pool(name="ad_w", bufs=1) as ad_w,
                tc.tile_pool(name="gelu", bufs=1) as gelu,
                tc.tile_pool(name="aps", bufs=3, space="PSUM") as aps,
                tc.tile_pool(name="fin", bufs=3) as fin,
            ):
                w1_sb = ad_w.tile([128, KD, DOWN], BF16)
                nc.sync.dma_start(out=w1_sb, in_=w1_r)
                w2_sb = ad_w.tile([128, 2, DIM], BF16)
                nc.sync.dma_start(out=w2_sb, in_=w2_r)
                g_T = gelu.tile([128, 2, NQE], BF16)
                for mt2, msz in ((0, 128), (1, 64)):
                    for qc in range(4):
                        n0 = qc * ACH
                        ps = aps.tile([128, ACH], F32, tag="aps1")
                        for kc in range(KD):
                            nc.tensor.matmul(ps[:msz], w1_sb[:, kc, mt2 * 128:mt2 * 128 + msz],
                                             xatt_T[:, kc, n0:n0 + ACH],
                                             start=(kc == 0), stop=(kc == KD - 1))
                        nc.scalar.activation(out=g_T[:msz, mt2, n0:n0 + ACH], in_=ps[:msz],
                                             func=AF.Gelu, bias=b1_sb[:msz, mt2:mt2 + 1],
                                             scale=1.0)
                for mt in range(KD):
                    for qc in range(4):
                        n0 = qc * ACH
                        ps = aps.tile([128, ACH], F32, tag="aps2")
                        nc.tensor.matmul(ps, w2_sb[:, 0, mt * 128:(mt + 1) * 128],
                                         g_T[:, 0, n0:n0 + ACH], start=True, stop=False)
                        nc.tensor.matmul(ps, w2_sb[0:64, 1, mt * 128:(mt + 1) * 128],
                                         g_T[0:64, 1, n0:n0 + ACH], start=False, stop=True)
                        # final = mlp + x_att (+ b2); host adds the raw residual
                        ft = fin.tile([128, ACH], F32, tag="ft")
                        nc.vector.tensor_scalar_add(out=ft, in0=ps,
                                                    scalar1=b2_sb[:, mt:mt + 1])
                        nc.vector.tensor_add(out=ft, in0=ft, in1=xatt_T[:, mt, n0:n0 + ACH])
                        nc.sync.dma_start(out=out_d[mt * 128:(mt + 1) * 128, n0:n0 + ACH],
                                          in_=ft)

    return nc


_NC_CACHE = None
_NC_FIXED = False


def _get_nc(fixed=False):
    """fixed=True applies the walrus wait-split (breaks CoreSim bookkeeping,
    required for NEFF compile)."""
    global _NC_CACHE, _NC_FIXED
    if _NC_CACHE is None:
        _NC_CACHE = build_nc()
    if fixed and not _NC_FIXED:
        _split_excess_waits(_NC_CACHE)
        _NC_FIXED = True
    return _NC_CACHE


def _prep_branch(params, branch):
    """Precompute per-branch device weights/tables (host-side folding)."""
    p = {k: np.asarray(v, np.float32) for k, v in params.items()}
    if branch == "s2t":
        pre = "s2t"
        gq, bq = p["ln_t_g"], p["ln_t_b"]          # q side = video stream
        gkv, bkv = p["ln_s_g"], p["ln_s_b"]        # kv side = audio stream
        space_q, temp_q = p[f"{pre}_vmae_space"], p[f"{pre}_vmae_temp"]
        space_k, temp_k = p[f"{pre}_clip_space"], p[f"{pre}_clip_temp"]
        ad = "ad_t"
    else:
        pre = "t2s"
        gq, bq = p["ln_s_g"], p["ln_s_b"]
        gkv, bkv = p["ln_t_g"], p["ln_t_b"]
        space_q, temp_q = p[f"{pre}_clip_space"], p[f"{pre}_clip_temp"]
        space_k, temp_k = p[f"{pre}_vmae_space"], p[f"{pre}_vmae_temp"]
        ad = "ad_s"

    qw, qb = p[f"{pre}_qw"], p[f"{pre}_qb"]
    kvw, kvb = p[f"{pre}_kvw"], p[f"{pre}_kvb"]
    kw, vw = kvw[:DIM], kvw[DIM:]
    kb, vb = kvb[:DIM], kvb[DIM:]
    pw, pb = p[f"{pre}_pw"], p[f"{pre}_pb"]
    w1, b1 = p[f"{ad}_w1"], p[f"{ad}_b1"]
    w2, b2 = p[f"{ad}_w2"], p[f"{ad}_b2"]

    # pos tables in attention token order (n-major, t-minor)
    pos_q = (space_q[:, None, :] + temp_q[None, :, :]).reshape(NQ, DIM)
    pos_k = (space_k[:, None, :] + temp_k[None, :, :]).reshape(NK, DIM)

    bf = lambda x: np.ascontiguousarray(x).astype(ml_dtypes.bfloat16)
    f32 = lambda x: np.ascontiguousarray(x, np.float32)

    Wq = (gq[:, None] * qw.T * SCALE)                       # [din, dout]
    posq_proj = ((bq + pos_q) @ qw.T + qb) * SCALE          # [NQ, dout]
    Wk = (gkv[:, None] * kw.T)
    posk_proj = (bkv + pos_k) @ kw.T + kb
    Wv = (gkv[:, None] * vw.T)                              # [din, 768]
    posv_proj = (bkv + pos_k) @ vw.T + vb                   # [NK, 768]

    Wv_aug = np.zeros((DIM, DV), np.float32)
    posv_aug = np.zeros((NK, DV), np.float32)
    for h in range(H):
        Wv_aug[:, h * 65:h * 65 + 64] = Wv[:, h * 64:(h + 1) * 64]
        posv_aug[:, h * 65:h * 65 + 64] = posv_proj[:, h * 64:(h + 1) * 64]
        posv_aug[:, h * 65 + 64] = 1.0

    b1_pad = np.zeros(256, np.float32)
    b1_pad[:DOWN] = b1
    w2_pad = np.zeros((256, DIM), np.float32)
    w2_pad[:DOWN] = w2.T

    return {
        "wq": bf(Wq), "wk": bf(Wk), "wv": bf(Wv_aug),
        "posq": bf(posq_proj.T), "posk": bf(posk_proj.T), "posv": bf(posv_aug),
        "wp": bf(pw.T), "pb": f32(pb),
        "w1": bf(w1.T), "b1": f32(b1_pad), "w2": bf(w2_pad), "b2": f32(b2),
        "gq": f32(gq), "bq": f32(bq),
        "ident": f32(np.eye(128, dtype=np.float32)),
    }


def kernel(s_x, t_x, params):
    s_x = np.asarray(s_x, np.float32)
    t_x = np.asarray(t_x, np.float32)
    nc = _get_nc(fixed=True)

    branch_inputs = {b: _prep_branch(params, b) for b in ("s2t", "t2s")}
    in_maps = []
    for c in range(8):
        branch = "s2t" if c < 4 else "t2s"
        b = c % 4
        bsl = slice(b * T, (b + 1) * T)
        m = dict(branch_inputs[branch])
        if branch == "s2t":
            xq = np.concatenate([
                np.ascontiguousarray(t_x[1:, bsl]).reshape(NQ, DIM),
                np.ascontiguousarray(t_x[0, bsl]),
            ], axis=0)
            xkv = np.ascontiguousarray(s_x[:, bsl]).reshape(NK, DIM)
        else:
            xq = np.concatenate([
                np.ascontiguousarray(s_x[:, bsl]).reshape(NQ, DIM),
                np.zeros((NEX, DIM), np.float32),
            ], axis=0)
            xkv = np.ascontiguousarray(t_x[1:, bsl]).reshape(NK, DIM)
        m["xq"] = np.ascontiguousarray(xq, np.float32)
        m["xkv"] = np.ascontiguousarray(xkv, np.float32)
        in_maps.append(m)

    res = run_bass_kernel_spmd(nc, in_maps, core_ids=list(range(8)))

    s_out = s_x.copy().reshape(NP, B, TS, DIM)
    t_out = t_x.copy().reshape(1 + NP, B, T, DIM)
    for c in range(8):
        o = np.asarray(res.results[c]["out"], np.float32).T  # [1576, 768]
        b = c % 4
        if c < 4:
            t_out[1:, b] += o[:NQ].reshape(NP, T, DIM)
            t_out[0, b] += o[NQ:NQE]
        else:
            s_out[:, b] += o[:NQ].reshape(NP, TS, DIM)
    return (s_out.reshape(NP, B * TS, DIM).astype(np.float32),
            t_out.reshape(1 + NP, B * T, DIM).astype(np.float32))


# revision 24
# speedup vs baseline: 1.2668x; 1.0055x over previous
"""Trainium2 Bass kernel for nn_Block_7722351198258 (dual cross-attention block).

Sharding: 8 cores = 2 branches (S2T video-queries / T2S audio-queries) x 4 batch
elements. Each core runs an identical SPMD program computing one branch-batch:
LN -> (+pos, folded) -> 12-head cross-attention (1568 q x 1568 kv tokens) ->
output projection -> bottleneck adapter. The host folds LN gamma into the
projection weights, beta/pos/bias into precomputed additive tables, shards
inputs, and adds the raw residual + reassembles outputs at the end.

Device layout: post-LN activations are feature-major [768, tok] (PE-transposed),
so projections, attention (scores_T [kk, qq]), softmax (exp on ScalarE with the
denominator from a ones-column fused into the v-projection), and the adapter
all run as bf16 matmuls with K=128 (K=64 scores run as 2-head row-tiled pairs).
Stage C interleaves scores/exp/attn-v per 128-row kv chunk so PE and ACT
pipeline; softmax normalization is deferred to the PSUM evacuation.
"""

import numpy as np
import ml_dtypes

import concourse.bass as bass
import concourse.mybir as mybir
import concourse.tile as tile
from concourse.bass_utils import run_bass_kernel_spmd

F32 = mybir.dt.float32
BF16 = mybir.dt.bfloat16
AF = mybir.ActivationFunctionType
ALU = mybir.AluOpType

DIM = 768
H = 12
HD = 64
SCALE = HD ** -0.5
T = 8
TS = 8
NP = 196
B = 4
DOWN = 192
EPS = 1e-5

NQ = NP * T            # 1568 query tokens (= kv tokens)
NK = NQ
NEX = 8                # extra rows (CLS for S2T / dummies for T2S)
NQE = NQ + NEX         # 1576 adapter tokens
KD = DIM // 128        # 6 feature chunks
NKT = 13               # kv token tiles: 12*128 + 32
NQT = 13               # q-side token tiles: 12*128 + 40
QCH = 392              # qq chunk (4*392 = 1568)
ACH = 394              # adapter N chunk (4*394 = 1576)
VCH = 390              # v-projection N chunk (2*390 = 780)
DV = H * (HD + 1)      # 780: v with per-head ones column

MAX_WAITS = 1


def _split_excess_waits(nc):
    """The walrus build here rejects instructions with >1 sem-wait; split
    extras onto preceding same-engine EventSemaphore carriers."""
    n_fixed = 0
    for f in nc.m.functions:
        for b in f.blocks:
            out = []
            changed = False
            for inst in b.instructions:
                si = inst.sync_info
                if si is not None and si.on_wait is not None and len(si.on_wait) > MAX_WAITS:
                    waits = list(si.on_wait)
                    keep = waits[-MAX_WAITS:]
                    extra = waits[:-MAX_WAITS]
                    ci = 0
                    while extra:
                        chunk, extra = extra[:MAX_WAITS], extra[MAX_WAITS:]
                        out.append(mybir.InstEventSemaphore(
                            name=f"{inst.name}_waitfix{ci}", engine=inst.engine,
                            ins=[], outs=[],
                            sync_info=mybir.SyncInfo(on_wait=chunk, on_update=[])))
                        ci += 1
                    inst.sync_info = mybir.SyncInfo(
                        on_wait=keep, on_update=list(si.on_update or []))
                    changed = True
                    n_fixed += 1
                out.append(inst)
            if changed:
                b.instructions = out
    return n_fixed


def _qtile_rows(it):
    return 128 if it < NQT - 1 else NQE - 128 * (NQT - 1)  # 40


def _ktile_rows(it):
    return 128 if it < NKT - 1 else NK - 128 * (NKT - 1)   # 32


def build_nc():
    nc = bass.Bass(trn_type="TRN2", name="xattn_block")

    xq_d = nc.dram_tensor("xq", [NQE, DIM], F32, kind="ExternalInput")
    xkv_d = nc.dram_tensor("xkv", [NK, DIM], F32, kind="ExternalInput")
    wq_d = nc.dram_tensor("wq", [DIM, DIM], BF16, kind="ExternalInput")
    wk_d = nc.dram_tensor("wk", [DIM, DIM], BF16, kind="ExternalInput")
    wv_d = nc.dram_tensor("wv", [DIM, DV], BF16, kind="ExternalInput")
    posq_d = nc.dram_tensor("posq", [DIM, NQ], BF16, kind="ExternalInput")
    posk_d = nc.dram_tensor("posk", [DIM, NK], BF16, kind="ExternalInput")
    posv_d = nc.dram_tensor("posv", [NK, DV], BF16, kind="ExternalInput")
    wp_d = nc.dram_tensor("wp", [DIM, DIM], BF16, kind="ExternalInput")
    pb_d = nc.dram_tensor("pb", [DIM], F32, kind="ExternalInput")
    w1_d = nc.dram_tensor("w1", [DIM, DOWN], BF16, kind="ExternalInput")
    b1_d = nc.dram_tensor("b1", [256], F32, kind="ExternalInput")
    w2_d = nc.dram_tensor("w2", [256, DIM], BF16, kind="ExternalInput")
    b2_d = nc.dram_tensor("b2", [DIM], F32, kind="ExternalInput")
    gq_d = nc.dram_tensor("gq", [DIM], F32, kind="ExternalInput")
    bq_d = nc.dram_tensor("bq", [DIM], F32, kind="ExternalInput")
    ident_d = nc.dram_tensor("ident", [128, 128], F32, kind="ExternalInput")
    out_d = nc.dram_tensor("out", [DIM, NQE], F32, kind="ExternalOutput")

    wq_r = wq_d.rearrange("(kc p) d -> p kc d", p=128)
    wk_r = wk_d.rearrange("(kc p) d -> p kc d", p=128)
    wv_r = wv_d.rearrange("(kc p) d -> p kc d", p=128)
    wp_r = wp_d.rearrange("(kc p) d -> p kc d", p=128)
    w1_r = w1_d.rearrange("(kc p) d -> p kc d", p=128)
    w2_r = w2_d.rearrange("(kc p) d -> p kc d", p=128)

    with tile.TileContext(nc) as tc:
        with (
            tc.tile_pool(name="const", bufs=1) as const,
            tc.tile_pool(name="xnt", bufs=1) as xnt,
            tc.tile_pool(name="qkv", bufs=1) as qkv,
            tc.tile_pool(name="attn_out", bufs=1) as attn_out,
        ):
            eps_t = const.tile([128, 1], F32)
            nc.vector.memset(eps_t, EPS)
            ones64 = const.tile([1, 64], BF16)
            nc.vector.memset(ones64, 1.0)
            ident = const.tile([128, 128], F32)
            nc.sync.dma_start(out=ident, in_=ident_d[:])
            gq_sb = const.tile([128, KD], F32)
            nc.sync.dma_start(out=gq_sb, in_=gq_d.rearrange("(c p) -> p c", p=128))
            bq_sb = const.tile([128, KD], F32)
            nc.sync.dma_start(out=bq_sb, in_=bq_d.rearrange("(c p) -> p c", p=128))
            pb_sb = const.tile([128, KD], F32)
            nc.sync.dma_start(out=pb_sb, in_=pb_d.rearrange("(c p) -> p c", p=128))
            b1_sb = const.tile([128, 2], F32)
            nc.sync.dma_start(out=b1_sb, in_=b1_d.rearrange("(c p) -> p c", p=128))
            b2_sb = const.tile([128, KD], F32)
            nc.sync.dma_start(out=b2_sb, in_=b2_d.rearrange("(c p) -> p c", p=128))

            # feature-major normalized activations (post-LN, pre-gamma)
            xnT_q = xnt.tile([128, KD, NQE], BF16)
            xnT_kv = xnt.tile([128, KD, NK], BF16)

            q_T = qkv.tile([128, KD, NQ], BF16)
            k_T = qkv.tile([128, KD, NK], BF16)
            v_sb = qkv.tile([128, NKT, DV], BF16)

            O_T = attn_out.tile([128, KD, NQ], BF16)

            # ------------- stages B+C interleaved: projections + attention ---
            # One shared PSUM pool: tag "ps" [128,1024] (2 banks) x3 bufs used
            # by both projection chunks and score pairs; tag "ops" (1 bank) x2
            # for attn-v accumulators / broadcast tiles. Projection chunks for
            # q/k of head-pair mt are injected into attention pair mt-1's
            # ACT-bound stretches to keep PE dense.
            with (
                tc.tile_pool(name="wload", bufs=3) as wload,
                tc.tile_pool(name="pos", bufs=3) as pos,
                tc.tile_pool(name="cps", bufs=2, space="PSUM") as cps,
                tc.tile_pool(name="opsum", bufs=4, space="PSUM") as opsum,
                tc.tile_pool(name="exps", bufs=16) as exps,
                tc.tile_pool(name="norm", bufs=4) as norm,
            ):
                wv_sb = wload.tile([128, KD, DV], BF16, tag="wv", bufs=1)
                nc.sync.dma_start(out=wv_sb, in_=wv_r)
                w_cur = {}

                def emit_qk_chunk(name, mt, qc):
                    if qc == 0:
                        wt = wload.tile([128, KD, 128], BF16, tag="w", bufs=3,
                                        name=f"w_{name}{mt}")
                        w_r = wq_r if name == "q" else wk_r
                        nc.sync.dma_start(out=wt, in_=w_r[:, :, mt * 128:(mt + 1) * 128])
                        w_cur[name] = wt
                    w_sb = w_cur[name]
                    pos_d = posq_d if name == "q" else posk_d
                    outT = q_T if name == "q" else k_T
                    src = xnT_q if name == "q" else xnT_kv
                    n0 = qc * QCH
                    ps = cps.tile([128, 1024], F32, tag="ps", name=f"ps_{name}{mt}_{qc}")
                    for kc in range(KD):
                        nc.tensor.matmul(ps[:, 0:QCH],
                                         w_sb[:, kc, :],
                                         src[:, kc, n0:n0 + QCH],
                                         start=(kc == 0), stop=(kc == KD - 1))
                    pt = pos.tile([128, QCH], BF16, tag="pt")
                    nc.sync.dma_start(out=pt, in_=pos_d[mt * 128:(mt + 1) * 128,
                                                        n0:n0 + QCH])
                    nc.vector.tensor_add(out=outT[:, mt, n0:n0 + QCH],
                                         in0=ps[:, 0:QCH], in1=pt)

                def emit_v_chunk(tt, vc):
                    rows = _ktile_rows(tt)
                    t0 = tt * 128
                    n0 = vc * VCH
                    ps = cps.tile([128, 1024], F32, tag="ps", name=f"ps_v{tt}_{vc}")
                    for kc in range(KD):
                        nc.tensor.matmul(ps[:rows, 0:VCH], xnT_kv[:, kc, t0:t0 + rows],
                                         wv_sb[:, kc, n0:n0 + VCH],
                                         start=(kc == 0), stop=(kc == KD - 1))
                    pt = pos.tile([128, VCH], BF16, tag="ptv")
                    nc.sync.dma_start(out=pt[:rows], in_=posv_d[t0:t0 + rows, n0:n0 + VCH])
                    nc.vector.tensor_add(out=v_sb[:rows, tt, n0:n0 + VCH],
                                         in0=ps[:rows, 0:VCH], in1=pt[:rows])

                def emit_ln_tile(ln, side, it):
                    """LN one 128-token tile and PE-transpose it into
                    xnT_q/xnT_kv (transposes share the 'ps' PSUM slots; ACT
                    evacuates; the xn scale runs on the otherwise idle
                    GpSimd)."""
                    x_d, rows_of, xnT = ((xq_d, _qtile_rows, xnT_q) if side == "q"
                                         else (xkv_d, _ktile_rows, xnT_kv))
                    rows = rows_of(it)
                    r0 = it * 128
                    raw = ln.tile([128, DIM], F32, tag="raw", name=f"raw_{side}{it}")
                    nc.sync.dma_start(out=raw[:rows], in_=x_d[r0:r0 + rows, :])
                    stats = ln.tile([128, 3, 6], F32, tag="stats", name=f"st_{side}{it}")
                    rawg = raw.rearrange("p (s f) -> p s f", s=3)
                    for s in range(3):
                        nc.vector.bn_stats(out=stats[:rows, s, :], in_=rawg[:rows, s, :])
                    mv = ln.tile([128, 2], F32, tag="mv", name=f"mv_{side}{it}")
                    nc.vector.bn_aggr(out=mv[:rows], in_=stats[:rows])
                    std = ln.tile([128, 1], F32, tag="std", name=f"sd_{side}{it}")
                    nc.scalar.activation(out=std[:rows], in_=mv[:rows, 1:2],
                                         func=AF.Sqrt, bias=eps_t[:rows], scale=1.0)
                    rstd = ln.tile([128, 1], F32, tag="rstd", name=f"rs_{side}{it}")
                    nc.vector.reciprocal(out=rstd[:rows], in_=std[:rows])
                    xn = ln.tile([128, DIM], F32, tag="xn", name=f"xn_{side}{it}")
                    nc.vector.tensor_scalar(
                        out=xn[:rows], in0=raw[:rows], scalar1=mv[:rows, 0:1],
                        scalar2=rstd[:rows], op0=ALU.subtract, op1=ALU.mult)
                    for c in range(KD):
                        tp = cps.tile([128, 1024], F32, tag="ps", name=f"tp_{side}{it}_{c}")
                        nc.tensor.transpose(tp[:, :rows], xn[:rows, c * 128:(c + 1) * 128],
                                            ident[:rows, :rows])
                        nc.scalar.copy(out=xnT[:, c, r0:r0 + rows], in_=tp[:, :rows])

                def emit_attnv(pr, q4, eS_tiles):
                    q0 = q4 * QCH
                    ops2 = []
                    for hi in range(2):
                        ops2.append(opsum.tile([65, QCH], F32, tag="ops",
                                               name=f"ops{pr}_{q4}_{hi}"))
                    for kc in range(NKT):
                        rows = _ktile_rows(kc)
                        for hi in range(2):
                            hh = 2 * pr + hi
                            nc.tensor.matmul(
                                ops2[hi], v_sb[:rows, kc, hh * 65:hh * 65 + 65],
                                eS_tiles[kc][:rows, hi, :],
                                start=(kc == 0), stop=(kc == NKT - 1))
                    return ops2

                def emit_norm(pr, q4, ops2):
                    q0 = q4 * QCH
                    for hi in range(2):
                        p0 = hi * 64
                        rinv = norm.tile([1, QCH], BF16, tag="rinv")
                        with nc.allow_low_precision(reason="softmax denom bf16 ok"):
                            nc.vector.reciprocal(out=rinv, in_=ops2[hi][64:65, :])
                        rbp = opsum.tile([65, QCH], F32, tag="ops",
                                         name=f"rbp{pr}_{q4}_{hi}")
                        nc.tensor.matmul(rbp[0:64, :], ones64, rinv)
                        rb = norm.tile([64, QCH], BF16, tag="rb")
                        nc.vector.tensor_copy(out=rb, in_=rbp[0:64, :])
                        nc.vector.tensor_mul(out=O_T[p0:p0 + 64, pr, q0:q0 + QCH],
                                             in0=ops2[hi][0:64, :], in1=rb)

                # stage A + prelude, interleaved for PE density: each kv tile's
                # LN/transpose immediately feeds its v-projection chunks; q
                # tiles feed pair-0 q/k projection chunks as token ranges
                # complete.
                import contextlib
                ln_stack = contextlib.ExitStack()
                ln = ln_stack.enter_context(tc.tile_pool(name="ln", bufs=4))
                for it in range(NKT):
                    emit_ln_tile(ln, "kv", it)
                    emit_v_chunk(it, 0)
                    emit_v_chunk(it, 1)
                qk0 = {3: [("q", 0, 0), ("k", 0, 0)], 6: [("q", 0, 1), ("k", 0, 1)],
                       9: [("q", 0, 2), ("k", 0, 2)],
                       12: [("q", 0, 3), ("k", 0, 3)]}
                for it in range(NQT):
                    emit_ln_tile(ln, "q", it)
                    for ch in qk0.get(it, []):
                        emit_qk_chunk(*ch)
                ln_stack.close()

                # software-pipelined units: unit u's scores/exp interleave with
                # unit u-1's attn-v, chunk by chunk; next pair's projection
                # chunks drop into the leftover ACT-bound slack
                units = [(pr, q4) for pr in range(KD) for q4 in range(4)]
                prev = None      # (pr, q4, eS_tiles)
                inject = []
                for pr, q4 in units:
                    if q4 == 0 and pr + 1 < KD:
                        inject = [("q", pr + 1, qc) for qc in range(4)] + \
                                 [("k", pr + 1, qc) for qc in range(4)]
                    q0 = q4 * QCH
                    eS_tiles = []
                    prev_ops = None
                    prev_norm = {}
                    if prev is not None:
                        prev_ops = []
                        for hi in range(2):
                            prev_ops.append(opsum.tile(
                                [65, QCH], F32, tag="ops",
                                name=f"ops{prev[0]}_{prev[1]}_{hi}"))
                    for kc in range(NKT):
                        rows = _ktile_rows(kc)
                        k0 = kc * 128
                        psQ = cps.tile([128, 1024], F32, tag="ps",
                                       name=f"psQ{pr}_{q4}_{kc}")
                        nc.tensor.matmul(psQ[:rows, 0:QCH],
                                         k_T[0:64, pr, k0:k0 + rows],
                                         q_T[0:64, pr, q0:q0 + QCH],
                                         tile_position=(0, 0))
                        nc.tensor.matmul(psQ[:rows, 512:512 + QCH],
                                         k_T[64:128, pr, k0:k0 + rows],
                                         q_T[64:128, pr, q0:q0 + QCH],
                                         tile_position=(64, 0))
                        eS = exps.tile([128, 2, QCH], BF16, tag="eS",
                                       name=f"eS{pr}_{q4}_{kc}")
                        psv = psQ.rearrange("p (h q) -> p h q", h=2)
                        nc.scalar.activation(out=eS[:rows], in_=psv[:rows, :, 0:QCH],
                                             func=AF.Exp)
                        eS_tiles.append(eS)
                        if prev is not None:
                            ppr, pq4, peS = prev
                            pq0 = pq4 * QCH
                            # prev's attn-v compressed into the first half of
                            # this sweep (2 kv chunks per step)
                            for j in (2 * kc, 2 * kc + 1):
                                if j < NKT:
                                    prows = _ktile_rows(j)
                                    for hi in range(2):
                                        hh = 2 * ppr + hi
                                        nc.tensor.matmul(
                                            prev_ops[hi],
                                            v_sb[:prows, j, hh * 65:hh * 65 + 65],
                                            peS[j][:prows, hi, :],
                                            start=(j == 0), stop=(j == NKT - 1))
                            # prev's normalization spread over the PE-light
                            # back half so the PE stream never blocks on it
                            if kc == 7:
                                for hi in range(2):
                                    rinv = norm.tile([1, QCH], BF16, tag="rinv",
                                                     name=f"rinv{ppr}_{pq4}_{hi}")
                                    with nc.allow_low_precision(reason="softmax denom"):
                                        nc.vector.reciprocal(out=rinv,
                                                             in_=prev_ops[hi][64:65, :])
                                    prev_norm[hi] = rinv
                            elif kc in (9, 10):
                                hi = kc - 9
                                rbp = opsum.tile([65, QCH], F32, tag="ops",
                                                 name=f"rbp{ppr}_{pq4}_{hi}")
                                nc.tensor.matmul(rbp[0:64, :], ones64, prev_norm[hi])
                                rb = norm.tile([64, QCH], BF16, tag="rb",
                                               name=f"rb{ppr}_{pq4}_{hi}")
                                nc.vector.tensor_copy(out=rb, in_=rbp[0:64, :])
                                prev_norm[2 + hi] = rb
                            elif kc in (11, 12):
                                hi = kc - 11
                                p0 = hi * 64
                                nc.vector.tensor_mul(
                                    out=O_T[p0:p0 + 64, ppr, pq0:pq0 + QCH],
                                    in0=prev_ops[hi][0:64, :], in1=prev_norm[2 + hi])
                        if kc in (6, 9) and inject:
                            emit_qk_chunk(*inject.pop(0))
                    prev = (pr, q4, eS_tiles)
                # drain the pipeline: last unit's attn-v + norm
                ops2 = emit_attnv(prev[0], prev[1], prev[2])
                emit_norm(prev[0], prev[1], ops2)

            # ---------------- stage D: output projection (+CLS LN) ----------------
            xatt_T = attn_out.tile([128, KD, NQE], BF16)
            with (
                tc.tile_pool(name="wp_pool", bufs=1) as wp_pool,
                tc.tile_pool(name="dps", bufs=3, space="PSUM") as dps,
            ):
                wp_sb = wp_pool.tile([128, KD, DIM], BF16)
                nc.sync.dma_start(out=wp_sb, in_=wp_r)
                for mt in range(KD):
                    for qc in range(4):
                        n0 = qc * QCH
                        ps = dps.tile([128, QCH], F32, tag="dps")
                        for kc in range(KD):
                            nc.tensor.matmul(ps, wp_sb[:, kc, mt * 128:(mt + 1) * 128],
                                             O_T[:, kc, n0:n0 + QCH],
                                             start=(kc == 0), stop=(kc == KD - 1))
                        nc.vector.tensor_scalar_add(
                            out=xatt_T[:, mt, n0:n0 + QCH], in0=ps,
                            scalar1=pb_sb[:, mt:mt + 1])
                    # CLS columns: full LN = xn*g + b
                    nc.vector.tensor_scalar(
                        out=xatt_T[:, mt, NQ:NQE], in0=xnT_q[:, mt, NQ:NQE],
                        scalar1=gq_sb[:, mt:mt + 1], scalar2=bq_sb[:, mt:mt + 1],
                        op0=ALU.mult, op1=ALU.add)

            # ---------------- stage E/F: adapter ----------------
            with (
                tc.tile_pool(name="ad_w", bufs=1) as ad_w,
                tc.tile_pool(name="gelu", bufs=1) as gelu,
                tc.tile_pool(name="aps", bufs=3, space="PSUM") as aps,
                tc.tile_pool(name="fin", bufs=3) as fin,
            ):
                w1_sb = ad_w.tile([128, KD, DOWN], BF16)
                nc.sync.dma_start(out=w1_sb, in_=w1_r)
                w2_sb = ad_w.tile([128, 2, DIM], BF16)
                nc.sync.dma_start(out=w2_sb, in_=w2_r)
                g_T = gelu.tile([128, 2, NQE], BF16)
                for mt2, msz in ((0, 128), (1, 64)):
                    for qc in range(4):
                        n0 = qc * ACH
                        ps = aps.tile([128, ACH], F32, tag="aps1")
                        for kc in range(KD):
                            nc.tensor.matmul(ps[:msz], w1_sb[:, kc, mt2 * 128:mt2 * 128 + msz],
                                             xatt_T[:, kc, n0:n0 + ACH],
                                             start=(kc == 0), stop=(kc == KD - 1))
                        nc.scalar.activation(out=g_T[:msz, mt2, n0:n0 + ACH], in_=ps[:msz],
                                             func=AF.Gelu, bias=b1_sb[:msz, mt2:mt2 + 1],
                                             scale=1.0)
                for mt in range(KD):
                    for qc in range(4):
                        n0 = qc * ACH
                        ps = aps.tile([128, ACH], F32, tag="aps2")
                        nc.tensor.matmul(ps, w2_sb[:, 0, mt * 128:(mt + 1) * 128],
                                         g_T[:, 0, n0:n0 + ACH], start=True, stop=False)
                        nc.tensor.matmul(ps, w2_sb[0:64, 1, mt * 128:(mt + 1) * 128],
                                         g_T[0:64, 1, n0:n0 + ACH], start=False, stop=True)
                        # final = mlp + x_att (+ b2); host adds the raw residual
                        ft = fin.tile([128, ACH], F32, tag="ft")
                        nc.vector.tensor_scalar_add(out=ft, in0=ps,
                                                    scalar1=b2_sb[:, mt:mt + 1])
                        nc.vector.tensor_add(out=ft, in0=ft, in1=xatt_T[:, mt, n0:n0 + ACH])
                        nc.sync.dma_start(out=out_d[mt * 128:(mt + 1) * 128, n0:n0 + ACH],
                                          in_=ft)

    return nc


_NC_CACHE = None
_NC_FIXED = False


def _get_nc(fixed=False):
    """fixed=True applies the walrus wait-split (breaks CoreSim bookkeeping,
    required for NEFF compile)."""
    global _NC_CACHE, _NC_FIXED
    if _NC_CACHE is None:
        _NC_CACHE = build_nc()
    if fixed and not _NC_FIXED:
        _split_excess_waits(_NC_CACHE)
        _NC_FIXED = True
    return _NC_CACHE


def _prep_branch(params, branch):
    """Precompute per-branch device weights/tables (host-side folding)."""
    p = {k: np.asarray(v, np.float32) for k, v in params.items()}
    if branch == "s2t":
        pre = "s2t"
        gq, bq = p["ln_t_g"], p["ln_t_b"]          # q side = video stream
        gkv, bkv = p["ln_s_g"], p["ln_s_b"]        # kv side = audio stream
        space_q, temp_q = p[f"{pre}_vmae_space"], p[f"{pre}_vmae_temp"]
        space_k, temp_k = p[f"{pre}_clip_space"], p[f"{pre}_clip_temp"]
        ad = "ad_t"
    else:
        pre = "t2s"
        gq, bq = p["ln_s_g"], p["ln_s_b"]
        gkv, bkv = p["ln_t_g"], p["ln_t_b"]
        space_q, temp_q = p[f"{pre}_clip_space"], p[f"{pre}_clip_temp"]
        space_k, temp_k = p[f"{pre}_vmae_space"], p[f"{pre}_vmae_temp"]
        ad = "ad_s"

    qw, qb = p[f"{pre}_qw"], p[f"{pre}_qb"]
    kvw, kvb = p[f"{pre}_kvw"], p[f"{pre}_kvb"]
    kw, vw = kvw[:DIM], kvw[DIM:]
    kb, vb = kvb[:DIM], kvb[DIM:]
    pw, pb = p[f"{pre}_pw"], p[f"{pre}_pb"]
    w1, b1 = p[f"{ad}_w1"], p[f"{ad}_b1"]
    w2, b2 = p[f"{ad}_w2"], p[f"{ad}_b2"]

    # pos tables in attention token order (n-major, t-minor)
    pos_q = (space_q[:, None, :] + temp_q[None, :, :]).reshape(NQ, DIM)
    pos_k = (space_k[:, None, :] + temp_k[None, :, :]).reshape(NK, DIM)

    bf = lambda x: np.ascontiguousarray(x).astype(ml_dtypes.bfloat16)
    f32 = lambda x: np.ascontiguousarray(x, np.float32)

    Wq = (gq[:, None] * qw.T * SCALE)                       # [din, dout]
    posq_proj = ((bq + pos_q) @ qw.T + qb) * SCALE          # [NQ, dout]
    Wk = (gkv[:, None] * kw.T)
    posk_proj = (bkv + pos_k) @ kw.T + kb
    Wv = (gkv[:, None] * vw.T)                              # [din, 768]
    posv_proj = (bkv + pos_k) @ vw.T + vb                   # [NK, 768]

    Wv_aug = np.zeros((DIM, DV), np.float32)
    posv_aug = np.zeros((NK, DV), np.float32)
    for h in range(H):
        Wv_aug[:, h * 65:h * 65 + 64] = Wv[:, h * 64:(h + 1) * 64]
        posv_aug[:, h * 65:h * 65 + 64] = posv_proj[:, h * 64:(h + 1) * 64]
        posv_aug[:, h * 65 + 64] = 1.0

    b1_pad = np.zeros(256, np.float32)
    b1_pad[:DOWN] = b1
    w2_pad = np.zeros((256, DIM), np.float32)
    w2_pad[:DOWN] = w2.T

    return {
        "wq": bf(Wq), "wk": bf(Wk), "wv": bf(Wv_aug),
        "posq": bf(posq_proj.T), "posk": bf(posk_proj.T), "posv": bf(posv_aug),
        "wp": bf(pw.T), "pb": f32(pb),
        "w1": bf(w1.T), "b1": f32(b1_pad), "w2": bf(w2_pad), "b2": f32(b2),
        "gq": f32(gq), "bq": f32(bq),
        "ident": f32(np.eye(128, dtype=np.float32)),
    }


def kernel(s_x, t_x, params):
    s_x = np.asarray(s_x, np.float32)
    t_x = np.asarray(t_x, np.float32)
    nc = _get_nc(fixed=True)

    branch_inputs = {b: _prep_branch(params, b) for b in ("s2t", "t2s")}
    in_maps = []
    for c in range(8):
        branch = "s2t" if c < 4 else "t2s"
        b = c % 4
        bsl = slice(b * T, (b + 1) * T)
        m = dict(branch_inputs[branch])
        if branch == "s2t":
            xq = np.concatenate([
                np.ascontiguousarray(t_x[1:, bsl]).reshape(NQ, DIM),
                np.ascontiguousarray(t_x[0, bsl]),
            ], axis=0)
            xkv = np.ascontiguousarray(s_x[:, bsl]).reshape(NK, DIM)
        else:
            xq = np.concatenate([
                np.ascontiguousarray(s_x[:, bsl]).reshape(NQ, DIM),
                np.zeros((NEX, DIM), np.float32),
            ], axis=0)
            xkv = np.ascontiguousarray(t_x[1:, bsl]).reshape(NK, DIM)
        m["xq"] = np.ascontiguousarray(xq, np.float32)
        m["xkv"] = np.ascontiguousarray(xkv, np.float32)
        in_maps.append(m)

    res = run_bass_kernel_spmd(nc, in_maps, core_ids=list(range(8)))

    s_out = s_x.copy().reshape(NP, B, TS, DIM)
    t_out = t_x.copy().reshape(1 + NP, B, T, DIM)
    for c in range(8):
        o = np.asarray(res.results[c]["out"], np.float32).T  # [1576, 768]
        b = c % 4
        if c < 4:
            t_out[1:, b] += o[:NQ].reshape(NP, T, DIM)
            t_out[0, b] += o[NQ:NQE]
        else:
            s_out[:, b] += o[:NQ].reshape(NP, TS, DIM)
    return (s_out.reshape(NP, B * TS, DIM).astype(np.float32),
            t_out.reshape(1 + NP, B * T, DIM).astype(np.float32))


# revision 26
# speedup vs baseline: 1.3184x; 1.0407x over previous
"""Trainium2 Bass kernel for nn_Block_7722351198258 (dual cross-attention block).

Sharding: 8 cores = 2 branches (S2T video-queries / T2S audio-queries) x 4 batch
elements. Each core runs an identical SPMD program computing one branch-batch:
LN -> (+pos, folded) -> 12-head cross-attention (1568 q x 1568 kv tokens) ->
output projection -> bottleneck adapter. The host folds LN gamma into the
projection weights, beta/pos/bias into precomputed additive tables, shards
inputs, and adds the raw residual + reassembles outputs at the end.

Device layout: post-LN activations are feature-major [768, tok] (PE-transposed),
so projections, attention (scores_T [kk, qq]), softmax (exp on ScalarE with the
denominator from a ones-column fused into the v-projection), and the adapter
all run as bf16 matmuls with K=128 (K=64 scores run as 2-head row-tiled pairs).
Stage C interleaves scores/exp/attn-v per 128-row kv chunk so PE and ACT
pipeline; softmax normalization is deferred to the PSUM evacuation.
"""

import numpy as np
import ml_dtypes

import concourse.bass as bass
import concourse.mybir as mybir
import concourse.tile as tile
from concourse.bass_utils import run_bass_kernel_spmd

F32 = mybir.dt.float32
BF16 = mybir.dt.bfloat16
AF = mybir.ActivationFunctionType
ALU = mybir.AluOpType

DIM = 768
H = 12
HD = 64
SCALE = HD ** -0.5
T = 8
TS = 8
NP = 196
B = 4
DOWN = 192
EPS = 1e-5

NQ = NP * T            # 1568 query tokens (= kv tokens)
NK = NQ
NEX = 8                # extra rows (CLS for S2T / dummies for T2S)
NQE = NQ + NEX         # 1576 adapter tokens
KD = DIM // 128        # 6 feature chunks
NKT = 13               # kv token tiles: 12*128 + 32
NQT = 13               # q-side token tiles: 12*128 + 40
QCH = 392              # qq chunk (4*392 = 1568)
ACH = 394              # adapter N chunk (4*394 = 1576)
VCH = 390              # v-projection N chunk (2*390 = 780)
DV = H * (HD + 1)      # 780: v with per-head ones column

MAX_WAITS = 1


def _split_excess_waits(nc):
    """The walrus build here rejects instructions with >1 sem-wait; split
    extras onto preceding same-engine EventSemaphore carriers."""
    n_fixed = 0
    for f in nc.m.functions:
        for b in f.blocks:
            out = []
            changed = False
            for inst in b.instructions:
                si = inst.sync_info
                if si is not None and si.on_wait is not None and len(si.on_wait) > MAX_WAITS:
                    waits = list(si.on_wait)
                    keep = waits[-MAX_WAITS:]
                    extra = waits[:-MAX_WAITS]
                    ci = 0
                    while extra:
                        chunk, extra = extra[:MAX_WAITS], extra[MAX_WAITS:]
                        out.append(mybir.InstEventSemaphore(
                            name=f"{inst.name}_waitfix{ci}", engine=inst.engine,
                            ins=[], outs=[],
                            sync_info=mybir.SyncInfo(on_wait=chunk, on_update=[])))
                        ci += 1
                    inst.sync_info = mybir.SyncInfo(
                        on_wait=keep, on_update=list(si.on_update or []))
                    changed = True
                    n_fixed += 1
                out.append(inst)
            if changed:
                b.instructions = out
    return n_fixed


def _qtile_rows(it):
    return 128 if it < NQT - 1 else NQE - 128 * (NQT - 1)  # 40


def _ktile_rows(it):
    return 128 if it < NKT - 1 else NK - 128 * (NKT - 1)   # 32


def build_nc():
    nc = bass.Bass(trn_type="TRN2", name="xattn_block")

    xq_d = nc.dram_tensor("xq", [NQE, DIM], F32, kind="ExternalInput")
    xkv_d = nc.dram_tensor("xkv", [NK, DIM], F32, kind="ExternalInput")
    wq_d = nc.dram_tensor("wq", [DIM, DIM], BF16, kind="ExternalInput")
    wk_d = nc.dram_tensor("wk", [DIM, DIM], BF16, kind="ExternalInput")
    wv_d = nc.dram_tensor("wv", [DIM, DV], BF16, kind="ExternalInput")
    posq_d = nc.dram_tensor("posq", [DIM, NQ], BF16, kind="ExternalInput")
    posk_d = nc.dram_tensor("posk", [DIM, NK], BF16, kind="ExternalInput")
    posv_d = nc.dram_tensor("posv", [NK, DV], BF16, kind="ExternalInput")
    wp_d = nc.dram_tensor("wp", [DIM, DIM], BF16, kind="ExternalInput")
    pb_d = nc.dram_tensor("pb", [DIM], F32, kind="ExternalInput")
    w1_d = nc.dram_tensor("w1", [DIM, DOWN], BF16, kind="ExternalInput")
    b1_d = nc.dram_tensor("b1", [256], F32, kind="ExternalInput")
    w2_d = nc.dram_tensor("w2", [256, DIM], BF16, kind="ExternalInput")
    b2_d = nc.dram_tensor("b2", [DIM], F32, kind="ExternalInput")
    gq_d = nc.dram_tensor("gq", [DIM], F32, kind="ExternalInput")
    bq_d = nc.dram_tensor("bq", [DIM], F32, kind="ExternalInput")
    ident_d = nc.dram_tensor("ident", [128, 128], F32, kind="ExternalInput")
    out_d = nc.dram_tensor("out", [DIM, NQE], F32, kind="ExternalOutput")

    wq_r = wq_d.rearrange("(kc p) d -> p kc d", p=128)
    wk_r = wk_d.rearrange("(kc p) d -> p kc d", p=128)
    wv_r = wv_d.rearrange("(kc p) d -> p kc d", p=128)
    wp_r = wp_d.rearrange("(kc p) d -> p kc d", p=128)
    w1_r = w1_d.rearrange("(kc p) d -> p kc d", p=128)
    w2_r = w2_d.rearrange("(kc p) d -> p kc d", p=128)

    with tile.TileContext(nc) as tc:
        with (
            tc.tile_pool(name="const", bufs=1) as const,
            tc.tile_pool(name="xnt", bufs=1) as xnt,
            tc.tile_pool(name="qkv", bufs=1) as qkv,
            tc.tile_pool(name="attn_out", bufs=1) as attn_out,
        ):
            eps_t = const.tile([128, 1], F32)
            nc.vector.memset(eps_t, EPS)
            ones64 = const.tile([1, 64], BF16)
            nc.vector.memset(ones64, 1.0)
            ident = const.tile([128, 128], F32)
            nc.sync.dma_start(out=ident, in_=ident_d[:])
            gq_sb = const.tile([128, KD], F32)
            nc.sync.dma_start(out=gq_sb, in_=gq_d.rearrange("(c p) -> p c", p=128))
            bq_sb = const.tile([128, KD], F32)
            nc.sync.dma_start(out=bq_sb, in_=bq_d.rearrange("(c p) -> p c", p=128))
            pb_sb = const.tile([128, KD], F32)
            nc.sync.dma_start(out=pb_sb, in_=pb_d.rearrange("(c p) -> p c", p=128))
            b1_sb = const.tile([128, 2], F32)
            nc.sync.dma_start(out=b1_sb, in_=b1_d.rearrange("(c p) -> p c", p=128))
            b2_sb = const.tile([128, KD], F32)
            nc.sync.dma_start(out=b2_sb, in_=b2_d.rearrange("(c p) -> p c", p=128))

            # feature-major normalized activations (post-LN, pre-gamma)
            xnT_q = xnt.tile([128, KD, NQE], BF16)
            xnT_kv = xnt.tile([128, KD, NK], BF16)

            q_T = qkv.tile([128, KD, NQ], BF16)
            k_T = qkv.tile([128, KD, NK], BF16)
            v_sb = qkv.tile([128, NKT, DV], BF16)

            O_T = attn_out.tile([128, KD, NQ], BF16)

            # ------------- stages B+C interleaved: projections + attention ---
            # One shared PSUM pool: tag "ps" [128,1024] (2 banks) x3 bufs used
            # by both projection chunks and score pairs; tag "ops" (1 bank) x2
            # for attn-v accumulators / broadcast tiles. Projection chunks for
            # q/k of head-pair mt are injected into attention pair mt-1's
            # ACT-bound stretches to keep PE dense.
            with (
                tc.tile_pool(name="wload", bufs=3) as wload,
                tc.tile_pool(name="pos", bufs=3) as pos,
                tc.tile_pool(name="cps", bufs=2, space="PSUM") as cps,
                tc.tile_pool(name="opsum", bufs=4, space="PSUM") as opsum,
                tc.tile_pool(name="exps", bufs=16) as exps,
                tc.tile_pool(name="norm", bufs=4) as norm,
            ):
                wv_sb = wload.tile([128, KD, DV], BF16, tag="wv", bufs=1)
                nc.sync.dma_start(out=wv_sb, in_=wv_r)
                w_cur = {}

                def emit_qk_chunk(name, mt, qc):
                    if qc == 0:
                        wt = wload.tile([128, KD, 128], BF16, tag="w", bufs=3,
                                        name=f"w_{name}{mt}")
                        w_r = wq_r if name == "q" else wk_r
                        nc.sync.dma_start(out=wt, in_=w_r[:, :, mt * 128:(mt + 1) * 128])
                        w_cur[name] = wt
                    w_sb = w_cur[name]
                    pos_d = posq_d if name == "q" else posk_d
                    outT = q_T if name == "q" else k_T
                    src = xnT_q if name == "q" else xnT_kv
                    n0 = qc * QCH
                    ps = cps.tile([128, 1024], F32, tag="ps", name=f"ps_{name}{mt}_{qc}")
                    for kc in range(KD):
                        nc.tensor.matmul(ps[:, 0:QCH],
                                         w_sb[:, kc, :],
                                         src[:, kc, n0:n0 + QCH],
                                         start=(kc == 0), stop=(kc == KD - 1))
                    pt = pos.tile([128, QCH], BF16, tag="pt")
                    nc.sync.dma_start(out=pt, in_=pos_d[mt * 128:(mt + 1) * 128,
                                                        n0:n0 + QCH])
                    nc.vector.tensor_add(out=outT[:, mt, n0:n0 + QCH],
                                         in0=ps[:, 0:QCH], in1=pt)

                def emit_v_chunk(tt, vc):
                    rows = _ktile_rows(tt)
                    t0 = tt * 128
                    n0 = vc * VCH
                    ps = cps.tile([128, 1024], F32, tag="ps", name=f"ps_v{tt}_{vc}")
                    for kc in range(KD):
                        nc.tensor.matmul(ps[:rows, 0:VCH], xnT_kv[:, kc, t0:t0 + rows],
                                         wv_sb[:, kc, n0:n0 + VCH],
                                         start=(kc == 0), stop=(kc == KD - 1))
                    pt = pos.tile([128, VCH], BF16, tag="ptv")
                    nc.sync.dma_start(out=pt[:rows], in_=posv_d[t0:t0 + rows, n0:n0 + VCH])
                    nc.vector.tensor_add(out=v_sb[:rows, tt, n0:n0 + VCH],
                                         in0=ps[:rows, 0:VCH], in1=pt[:rows])

                def emit_ln_tile(ln, side, it):
                    """LN one 128-token tile and PE-transpose it into
                    xnT_q/xnT_kv (transposes share the 'ps' PSUM slots; ACT
                    evacuates; the xn scale runs on the otherwise idle
                    GpSimd)."""
                    x_d, rows_of, xnT = ((xq_d, _qtile_rows, xnT_q) if side == "q"
                                         else (xkv_d, _ktile_rows, xnT_kv))
                    rows = rows_of(it)
                    r0 = it * 128
                    raw = ln.tile([128, DIM], F32, tag="raw", name=f"raw_{side}{it}")
                    nc.sync.dma_start(out=raw[:rows], in_=x_d[r0:r0 + rows, :])
                    stats = ln.tile([128, 3, 6], F32, tag="stats", name=f"st_{side}{it}")
                    rawg = raw.rearrange("p (s f) -> p s f", s=3)
                    for s in range(3):
                        nc.vector.bn_stats(out=stats[:rows, s, :], in_=rawg[:rows, s, :])
                    mv = ln.tile([128, 2], F32, tag="mv", name=f"mv_{side}{it}")
                    nc.vector.bn_aggr(out=mv[:rows], in_=stats[:rows])
                    std = ln.tile([128, 1], F32, tag="std", name=f"sd_{side}{it}")
                    nc.scalar.activation(out=std[:rows], in_=mv[:rows, 1:2],
                                         func=AF.Sqrt, bias=eps_t[:rows], scale=1.0)
                    rstd = ln.tile([128, 1], F32, tag="rstd", name=f"rs_{side}{it}")
                    nc.vector.reciprocal(out=rstd[:rows], in_=std[:rows])
                    xn = ln.tile([128, DIM], F32, tag="xn", name=f"xn_{side}{it}")
                    nc.vector.tensor_scalar(
                        out=xn[:rows], in0=raw[:rows], scalar1=mv[:rows, 0:1],
                        scalar2=rstd[:rows], op0=ALU.subtract, op1=ALU.mult)
                    for c in range(KD):
                        tp = cps.tile([128, 1024], F32, tag="ps", name=f"tp_{side}{it}_{c}")
                        nc.tensor.transpose(tp[:, :rows], xn[:rows, c * 128:(c + 1) * 128],
                                            ident[:rows, :rows])
                        nc.scalar.copy(out=xnT[:, c, r0:r0 + rows], in_=tp[:, :rows])

                def emit_attnv(pr, q4, eS_tiles):
                    q0 = q4 * QCH
                    ops2 = []
                    for hi in range(2):
                        ops2.append(opsum.tile([65, QCH], F32, tag="ops",
                                               name=f"ops{pr}_{q4}_{hi}"))
                    for kc in range(NKT):
                        rows = _ktile_rows(kc)
                        for hi in range(2):
                            hh = 2 * pr + hi
                            nc.tensor.matmul(
                                ops2[hi], v_sb[:rows, kc, hh * 65:hh * 65 + 65],
                                eS_tiles[kc][:rows, hi, :],
                                start=(kc == 0), stop=(kc == NKT - 1))
                    return ops2

                def emit_norm(pr, q4, ops2):
                    q0 = q4 * QCH
                    for hi in range(2):
                        p0 = hi * 64
                        rinv = norm.tile([1, QCH], BF16, tag="rinv")
                        with nc.allow_low_precision(reason="softmax denom bf16 ok"):
                            nc.vector.reciprocal(out=rinv, in_=ops2[hi][64:65, :])
                        rbp = opsum.tile([65, QCH], F32, tag="ops",
                                         name=f"rbp{pr}_{q4}_{hi}")
                        nc.tensor.matmul(rbp[0:64, :], ones64, rinv)
                        rb = norm.tile([64, QCH], BF16, tag="rb")
                        nc.vector.tensor_copy(out=rb, in_=rbp[0:64, :])
                        nc.vector.tensor_mul(out=O_T[p0:p0 + 64, pr, q0:q0 + QCH],
                                             in0=ops2[hi][0:64, :], in1=rb)

                # stage A + prelude, interleaved for PE density: each kv tile's
                # LN/transpose immediately feeds its v-projection chunks; q
                # tiles feed pair-0 q/k projection chunks as token ranges
                # complete.
                import contextlib
                ln_stack = contextlib.ExitStack()
                ln = ln_stack.enter_context(tc.tile_pool(name="ln", bufs=4))
                for it in range(NKT):
                    emit_ln_tile(ln, "kv", it)
                    emit_v_chunk(it, 0)
                    emit_v_chunk(it, 1)
                qk0 = {3: [("q", 0, 0), ("k", 0, 0)], 6: [("q", 0, 1), ("k", 0, 1)],
                       9: [("q", 0, 2), ("k", 0, 2)],
                       12: [("q", 0, 3), ("k", 0, 3)]}
                for it in range(NQT):
                    emit_ln_tile(ln, "q", it)
                    for ch in qk0.get(it, []):
                        emit_qk_chunk(*ch)
                ln_stack.close()

                # software-pipelined units: unit u's scores/exp interleave with
                # unit u-1's attn-v, chunk by chunk; next pair's projection
                # chunks drop into the leftover ACT-bound slack
                units = [(pr, q4) for pr in range(KD) for q4 in range(4)]
                prev = None      # (pr, q4, eS_tiles)
                inject = []
                for pr, q4 in units:
                    if q4 == 0 and pr + 1 < KD:
                        inject = [("q", pr + 1, qc) for qc in range(4)] + \
                                 [("k", pr + 1, qc) for qc in range(4)]
                    q0 = q4 * QCH
                    eS_tiles = []
                    prev_ops = None
                    prev_norm = {}
                    if prev is not None:
                        prev_ops = []
                        for hi in range(2):
                            prev_ops.append(opsum.tile(
                                [65, QCH], F32, tag="ops",
                                name=f"ops{prev[0]}_{prev[1]}_{hi}"))
                    # unit-boundary PE filler: the exp stream lags the score
                    # stream here, so give PE an independent projection chunk
                    if inject:
                        emit_qk_chunk(*inject.pop(0))
                    for kc in range(NKT):
                        rows = _ktile_rows(kc)
                        k0 = kc * 128
                        psQ = cps.tile([128, 1024], F32, tag="ps",
                                       name=f"psQ{pr}_{q4}_{kc}")
                        nc.tensor.matmul(psQ[:rows, 0:QCH],
                                         k_T[0:64, pr, k0:k0 + rows],
                                         q_T[0:64, pr, q0:q0 + QCH],
                                         tile_position=(0, 0))
                        nc.tensor.matmul(psQ[:rows, 512:512 + QCH],
                                         k_T[64:128, pr, k0:k0 + rows],
                                         q_T[64:128, pr, q0:q0 + QCH],
                                         tile_position=(64, 0))
                        eS = exps.tile([128, 2, QCH], BF16, tag="eS",
                                       name=f"eS{pr}_{q4}_{kc}")
                        psv = psQ.rearrange("p (h q) -> p h q", h=2)
                        nc.scalar.activation(out=eS[:rows], in_=psv[:rows, :, 0:QCH],
                                             func=AF.Exp)
                        eS_tiles.append(eS)
                        if prev is not None:
                            ppr, pq4, peS = prev
                            pq0 = pq4 * QCH
                            # prev's attn-v compressed into the first half of
                            # this sweep (2 kv chunks per step)
                            for j in (2 * kc, 2 * kc + 1):
                                if j < NKT:
                                    prows = _ktile_rows(j)
                                    for hi in range(2):
                                        hh = 2 * ppr + hi
                                        nc.tensor.matmul(
                                            prev_ops[hi],
                                            v_sb[:prows, j, hh * 65:hh * 65 + 65],
                                            peS[j][:prows, hi, :],
                                            start=(j == 0), stop=(j == NKT - 1))
                            # prev's normalization spread over the PE-light
                            # back half so the PE stream never blocks on it
                            if kc == 7:
                                for hi in range(2):
                                    rinv = norm.tile([1, QCH], BF16, tag="rinv",
                                                     name=f"rinv{ppr}_{pq4}_{hi}")
                                    with nc.allow_low_precision(reason="softmax denom"):
                                        nc.vector.reciprocal(out=rinv,
                                                             in_=prev_ops[hi][64:65, :])
                                    prev_norm[hi] = rinv
                            elif kc in (9, 10):
                                hi = kc - 9
                                rbp = opsum.tile([65, QCH], F32, tag="ops",
                                                 name=f"rbp{ppr}_{pq4}_{hi}")
                                nc.tensor.matmul(rbp[0:64, :], ones64, prev_norm[hi])
                                rb = norm.tile([64, QCH], BF16, tag="rb",
                                               name=f"rb{ppr}_{pq4}_{hi}")
                                nc.vector.tensor_copy(out=rb, in_=rbp[0:64, :])
                                prev_norm[2 + hi] = rb
                            elif kc in (11, 12):
                                hi = kc - 11
                                p0 = hi * 64
                                nc.vector.tensor_mul(
                                    out=O_T[p0:p0 + 64, ppr, pq0:pq0 + QCH],
                                    in0=prev_ops[hi][0:64, :], in1=prev_norm[2 + hi])
                        if kc == 1 and inject:
                            emit_qk_chunk(*inject.pop(0))
                    prev = (pr, q4, eS_tiles)
                # drain the pipeline: last unit's attn-v + norm
                ops2 = emit_attnv(prev[0], prev[1], prev[2])
                emit_norm(prev[0], prev[1], ops2)

            # ---------------- stage D: output projection (+CLS LN) ----------------
            xatt_T = attn_out.tile([128, KD, NQE], BF16)
            with (
                tc.tile_pool(name="wp_pool", bufs=1) as wp_pool,
                tc.tile_pool(name="dps", bufs=3, space="PSUM") as dps,
            ):
                wp_sb = wp_pool.tile([128, KD, DIM], BF16)
                nc.sync.dma_start(out=wp_sb, in_=wp_r)
                for mt in range(KD):
                    for qc in range(4):
                        n0 = qc * QCH
                        ps = dps.tile([128, QCH], F32, tag="dps")
                        for kc in range(KD):
                            nc.tensor.matmul(ps, wp_sb[:, kc, mt * 128:(mt + 1) * 128],
                                             O_T[:, kc, n0:n0 + QCH],
                                             start=(kc == 0), stop=(kc == KD - 1))
                        nc.vector.tensor_scalar_add(
                            out=xatt_T[:, mt, n0:n0 + QCH], in0=ps,
                            scalar1=pb_sb[:, mt:mt + 1])
                    # CLS columns: full LN = xn*g + b
                    nc.vector.tensor_scalar(
                        out=xatt_T[:, mt, NQ:NQE], in0=xnT_q[:, mt, NQ:NQE],
                        scalar1=gq_sb[:, mt:mt + 1], scalar2=bq_sb[:, mt:mt + 1],
                        op0=ALU.mult, op1=ALU.add)

            # ---------------- stage E/F: adapter ----------------
            with (
                tc.tile_pool(name="ad_w", bufs=1) as ad_w,
                tc.tile_pool(name="gelu", bufs=1) as gelu,
                tc.tile_pool(name="aps", bufs=3, space="PSUM") as aps,
                tc.tile_pool(name="fin", bufs=3) as fin,
            ):
                w1_sb = ad_w.tile([128, KD, DOWN], BF16)
                nc.sync.dma_start(out=w1_sb, in_=w1_r)
                w2_sb = ad_w.tile([128, 2, DIM], BF16)
                nc.sync.dma_start(out=w2_sb, in_=w2_r)
                g_T = gelu.tile([128, 2, NQE], BF16)
                for mt2, msz in ((0, 128), (1, 64)):
                    for qc in range(4):
                        n0 = qc * ACH
                        ps = aps.tile([128, ACH], F32, tag="aps1")
                        for kc in range(KD):
                            nc.tensor.matmul(ps[:msz], w1_sb[:, kc, mt2 * 128:mt2 * 128 + msz],
                                             xatt_T[:, kc, n0:n0 + ACH],
                                             start=(kc == 0), stop=(kc == KD - 1))
                        nc.scalar.activation(out=g_T[:msz, mt2, n0:n0 + ACH], in_=ps[:msz],
                                             func=AF.Gelu, bias=b1_sb[:msz, mt2:mt2 + 1],
                                             scale=1.0)
                for mt in range(KD):
                    for qc in range(4):
                        n0 = qc * ACH
                        ps = aps.tile([128, ACH], F32, tag="aps2")
                        nc.tensor.matmul(ps, w2_sb[:, 0, mt * 128:(mt + 1) * 128],
                                         g_T[:, 0, n0:n0 + ACH], start=True, stop=False)
                        nc.tensor.matmul(ps, w2_sb[0:64, 1, mt * 128:(mt + 1) * 128],
                                         g_T[0:64, 1, n0:n0 + ACH], start=False, stop=True)
                        # final = mlp + x_att (+ b2); host adds the raw residual
                        ft = fin.tile([128, ACH], F32, tag="ft")
                        nc.vector.tensor_scalar_add(out=ft, in0=ps,
                                                    scalar1=b2_sb[:, mt:mt + 1])
                        nc.vector.tensor_add(out=ft, in0=ft, in1=xatt_T[:, mt, n0:n0 + ACH])
                        nc.sync.dma_start(out=out_d[mt * 128:(mt + 1) * 128, n0:n0 + ACH],
                                          in_=ft)

    return nc


_NC_CACHE = None
_NC_FIXED = False


def _get_nc(fixed=False):
    """fixed=True applies the walrus wait-split (breaks CoreSim bookkeeping,
    required for NEFF compile)."""
    global _NC_CACHE, _NC_FIXED
    if _NC_CACHE is None:
        _NC_CACHE = build_nc()
    if fixed and not _NC_FIXED:
        _split_excess_waits(_NC_CACHE)
        _NC_FIXED = True
    return _NC_CACHE


def _prep_branch(params, branch):
    """Precompute per-branch device weights/tables (host-side folding)."""
    p = {k: np.asarray(v, np.float32) for k, v in params.items()}
    if branch == "s2t":
        pre = "s2t"
        gq, bq = p["ln_t_g"], p["ln_t_b"]          # q side = video stream
        gkv, bkv = p["ln_s_g"], p["ln_s_b"]        # kv side = audio stream
        space_q, temp_q = p[f"{pre}_vmae_space"], p[f"{pre}_vmae_temp"]
        space_k, temp_k = p[f"{pre}_clip_space"], p[f"{pre}_clip_temp"]
        ad = "ad_t"
    else:
        pre = "t2s"
        gq, bq = p["ln_s_g"], p["ln_s_b"]
        gkv, bkv = p["ln_t_g"], p["ln_t_b"]
        space_q, temp_q = p[f"{pre}_clip_space"], p[f"{pre}_clip_temp"]
        space_k, temp_k = p[f"{pre}_vmae_space"], p[f"{pre}_vmae_temp"]
        ad = "ad_s"

    qw, qb = p[f"{pre}_qw"], p[f"{pre}_qb"]
    kvw, kvb = p[f"{pre}_kvw"], p[f"{pre}_kvb"]
    kw, vw = kvw[:DIM], kvw[DIM:]
    kb, vb = kvb[:DIM], kvb[DIM:]
    pw, pb = p[f"{pre}_pw"], p[f"{pre}_pb"]
    w1, b1 = p[f"{ad}_w1"], p[f"{ad}_b1"]
    w2, b2 = p[f"{ad}_w2"], p[f"{ad}_b2"]

    # pos tables in attention token order (n-major, t-minor)
    pos_q = (space_q[:, None, :] + temp_q[None, :, :]).reshape(NQ, DIM)
    pos_k = (space_k[:, None, :] + temp_k[None, :, :]).reshape(NK, DIM)

    bf = lambda x: np.ascontiguousarray(x).astype(ml_dtypes.bfloat16)
    f32 = lambda x: np.ascontiguousarray(x, np.float32)

    Wq = (gq[:, None] * qw.T * SCALE)                       # [din, dout]
    posq_proj = ((bq + pos_q) @ qw.T + qb) * SCALE          # [NQ, dout]
    Wk = (gkv[:, None] * kw.T)
    posk_proj = (bkv + pos_k) @ kw.T + kb
    Wv = (gkv[:, None] * vw.T)                              # [din, 768]
    posv_proj = (bkv + pos_k) @ vw.T + vb                   # [NK, 768]

    Wv_aug = np.zeros((DIM, DV), np.float32)
    posv_aug = np.zeros((NK, DV), np.float32)
    for h in range(H):
        Wv_aug[:, h * 65:h * 65 + 64] = Wv[:, h * 64:(h + 1) * 64]
        posv_aug[:, h * 65:h * 65 + 64] = posv_proj[:, h * 64:(h + 1) * 64]
        posv_aug[:, h * 65 + 64] = 1.0

    b1_pad = np.zeros(256, np.float32)
    b1_pad[:DOWN] = b1
    w2_pad = np.zeros((256, DIM), np.float32)
    w2_pad[:DOWN] = w2.T

    return {
        "wq": bf(Wq), "wk": bf(Wk), "wv": bf(Wv_aug),
        "posq": bf(posq_proj.T), "posk": bf(posk_proj.T), "posv": bf(posv_aug),
        "wp": bf(pw.T), "pb": f32(pb),
        "w1": bf(w1.T), "b1": f32(b1_pad), "w2": bf(w2_pad), "b2": f32(b2),
        "gq": f32(gq), "bq": f32(bq),
        "ident": f32(np.eye(128, dtype=np.float32)),
    }


def kernel(s_x, t_x, params):
    s_x = np.asarray(s_x, np.float32)
    t_x = np.asarray(t_x, np.float32)
    nc = _get_nc(fixed=True)

    branch_inputs = {b: _prep_branch(params, b) for b in ("s2t", "t2s")}
    in_maps = []
    for c in range(8):
        branch = "s2t" if c < 4 else "t2s"
        b = c % 4
        bsl = slice(b * T, (b + 1) * T)
        m = dict(branch_inputs[branch])
        if branch == "s2t":
            xq = np.concatenate([
                np.ascontiguousarray(t_x[1:, bsl]).reshape(NQ, DIM),
                np.ascontiguousarray(t_x[0, bsl]),
            ], axis=0)
            xkv = np.ascontiguousarray(s_x[:, bsl]).reshape(NK, DIM)
        else:
            xq = np.concatenate([
                np.ascontiguousarray(s_x[:, bsl]).reshape(NQ, DIM),
                np.zeros((NEX, DIM), np.float32),
            ], axis=0)
            xkv = np.ascontiguousarray(t_x[1:, bsl]).reshape(NK, DIM)
        m["xq"] = np.ascontiguousarray(xq, np.float32)
        m["xkv"] = np.ascontiguousarray(xkv, np.float32)
        in_maps.append(m)

    res = run_bass_kernel_spmd(nc, in_maps, core_ids=list(range(8)))

    s_out = s_x.copy().reshape(NP, B, TS, DIM)
    t_out = t_x.copy().reshape(1 + NP, B, T, DIM)
    for c in range(8):
        o = np.asarray(res.results[c]["out"], np.float32).T  # [1576, 768]
        b = c % 4
        if c < 4:
            t_out[1:, b] += o[:NQ].reshape(NP, T, DIM)
            t_out[0, b] += o[NQ:NQE]
        else:
            s_out[:, b] += o[:NQ].reshape(NP, TS, DIM)
    return (s_out.reshape(NP, B * TS, DIM).astype(np.float32),
            t_out.reshape(1 + NP, B * T, DIM).astype(np.float32))
